# revision 27
# baseline (speedup 1.0000x reference)
"""GCNAggregator Trainium2 Bass kernel.

out[i] = (sum_{e: seg[e]==i} features[neighbor_idx[e]] + features[i]) / (deg_i + 1)

Strategy (8 NeuronCores, SPMD):
  - dma_gather indices are int16 (max 32767) but the table has 50000 rows,
    so every gathered row is classed L (row < 32768, gathered from the
    table base) or H (row >= 32768, gathered from an offset view). Each
    core is given a contiguous slice of the LOW dest nodes and a
    contiguous slice of the HIGH dest nodes, edge-balanced within each
    class, so per-core L/H gather totals match across cores to ~0.1%
    (one self-loop edge per node is folded in, and self rows are all-L on
    low nodes / all-H on high nodes).
  - Features ride as a bf16 table (512B/row -- the cost model's DMA
    sweet spot). Rel err ~3e-3 end-to-end, well inside the 2e-2 gate.
  - Per core, dest nodes are packed into 51 "slots" of <=128 consecutive
    nodes. The L rows of all slots form one dense stream in dest order
    (H likewise): no per-slot alignment padding. Slot boundaries are
    anchored to shared cumulative targets so every core's slot-g stream
    interval lands within a tile or two of the same place.
  - Streams are gathered with full 1024-descriptor dma_gather calls
    (the SWDGE ring caps at 1024 descriptors per call; bigger rings wedge
    the device) into circular SBUF rings of 128-row tiles, ~1us fixed
    Pool-engine cost per call. Gather calls ignore slot boundaries.
  - Segment-sum per slot on the tensor engine: for each stream tile
    overlapping the slot's interval on ANY core,
        psum[128 nodes, 256] += onehot[128 rows, 128 nodes]^T @ ring[tile]
    with the bf16 one-hot built on device by is_equal(iota, srel) from
    per-(tile,slot) relative dest ids; rows of a boundary tile that
    belong to the neighboring slot carry srel -1 there and are picked up
    by that slot's own matmul over the same tile. PSUM accumulates fp32.
  - Finalize per slot: out = psum * 1/(deg+1) (bf16), DMA out.
  - Gather indices ride the wire once as [16, ni16] f32 and are
    replicated on device to the 128-partition wrapped layout the SWDGE
    ucode requires (8 gpsimd cores read their own partition group) via
    one-hot f32 matmuls + Activation-engine PSUM->int16 copies, produced
    just-in-time between slot chains.
  - Engine budget per core (TimelineSim): DMA ~309us (97% busy, the
    bottleneck: 208k gather descriptors x 512B at 360GB/s aggregate),
    Pool ~273us, PE ~235us, DVE ~203us, Act ~17us.

The host only computes integer index metadata (shard boundaries, stream
index layouts, relative segment ids, degrees); all floating point work
(gather, segment sum, normalize) runs on device.
"""

import os as _os
import sys

import numpy as np

try:
    import concourse  # noqa: F401
except ImportError:  # pragma: no cover
    sys.path.insert(0, "/opt/trn_rl_repo")

from contextlib import ExitStack

import concourse.mybir as mybir
from concourse import bacc, bass_utils, tile

N_NODES = 50000
N_EDGES = 1_600_000
D = 256
N_CORES = 8
SPLIT = 32768       # int16 gather-index window
NSA = 33            # slots covering the core's low-node slice
NSB = 18            # slots covering the core's high-node slice
NS = NSA + NSB

_PROGRAM_CACHE: dict = {}
LAST_NC = None  # exposed for test harness introspection (TimelineSim)

MAX_GATHER = 1024   # SWDGE descriptor ring capacity per dma_gather call
RING_L = 96         # L-stream SBUF ring, in 128-row tiles (multiple of 8)
RING_H = 56         # H-stream ring


def _pad_calls(rows):
    return -(-rows // MAX_GATHER) * MAX_GATHER


def _build_program(spans):
    """Build + compile the (uniform-across-cores, SPMD) per-core program.

    spans = (aL, bL, aH, bH): per-slot stream-tile intervals, the union
    over the 8 cores of each slot's L/H stream coverage. The program
    matmuls every (slot, tile) pair in these intervals; per-core srel
    data masks which rows of the tile actually belong to the slot.
    """
    aL, bL, aH, bH = (list(v) for v in spans)
    nt_l, nt_h = max(bL), max(bH)
    rows_l, rows_h = _pad_calls(nt_l * 128), _pad_calls(nt_h * 128)
    ni16 = (rows_l + rows_h) // 16
    ncol = sum(b - a for a, b in zip(aL, bL)) + sum(
        b - a for a, b in zip(aH, bH)
    )

    nc = bacc.Bacc(
        "TRN2", target_bir_lowering=False, debug=False, num_devices=N_CORES,
    )

    feat_d = nc.dram_tensor(
        "featb", (N_NODES, D), mybir.dt.bfloat16, kind="ExternalInput"
    ).ap()
    gidx_d = nc.dram_tensor(
        "gidxf", (16, ni16), mybir.dt.float32, kind="ExternalInput"
    ).ap()
    rep_d = nc.dram_tensor(
        "repmat", (16, 128), mybir.dt.float32, kind="ExternalInput"
    ).ap()
    srel_d = nc.dram_tensor(
        "srel", (128, ncol), mybir.dt.bfloat16, kind="ExternalInput"
    ).ap()
    cnt1_d = nc.dram_tensor(
        "cnt1", (128, NS), mybir.dt.float32, kind="ExternalInput"
    ).ap()
    out_d = nc.dram_tensor(
        "out", (NS * 128, D), mybir.dt.bfloat16, kind="ExternalOutput"
    ).ap()

    feat_lo = feat_d[0:SPLIT, :]
    feat_hi = feat_d[SPLIT:N_NODES, :]

    with tile.TileContext(nc) as tc:
        with ExitStack() as ctx:
            ob = int(_os.environ.get("OH_BUFS", "4"))
            fb = int(_os.environ.get("FIN_BUFS", "3"))
            pb = int(_os.environ.get("PSUM_BUFS", "4"))
            const_pool = ctx.enter_context(tc.tile_pool(name="const", bufs=1))
            oh_pool = ctx.enter_context(tc.tile_pool(name="oh", bufs=ob))
            fin_pool = ctx.enter_context(tc.tile_pool(name="fin", bufs=fb))
            psum_pool = ctx.enter_context(
                tc.tile_pool(name="psum", bufs=pb, space="PSUM")
            )

            # gather indices ride the wire once as [16, ni16] f32 (exact for
            # idx < 2^24) and are replicated to the 128-partition wrapped
            # layout the SWDGE ucode needs via one-hot f32 matmuls on the
            # (otherwise idle-at-start) tensor engine, with PSUM->SBUF int16
            # copies on the idle Activation engine. Chunks are produced
            # just-in-time between slot accumulation chains, ahead of the
            # gather calls that read them.
            gidx_sb = const_pool.tile([128, ni16], mybir.dt.int16)
            gidxf_sb = const_pool.tile([16, ni16], mybir.dt.float32)
            rep_sb = const_pool.tile([16, 128], mybir.dt.float32)
            nc.sync.dma_start(rep_sb[:], rep_d[:])
            n_ld = 8
            ld_bounds = [ni16 * i // n_ld for i in range(n_ld + 1)]
            for a, b in zip(ld_bounds[:-1], ld_bounds[1:]):
                if b > a:
                    nc.sync.dma_start(gidxf_sb[:, a:b], gidx_d[:, a:b])
            rep_pool = ctx.enter_context(
                tc.tile_pool(name="rpsum", bufs=2, space="PSUM")
            )
            # independent replication cursors for the L and H col regions
            rep_state = {"L": 0, "H": rows_l // 16}
            rep_end = {"L": rows_l // 16, "H": ni16}

            def rep_to(region, col_need):
                while rep_state[region] < min(col_need, rep_end[region]):
                    a = rep_state[region]
                    b = min(a + 512, rep_end[region])
                    rp = rep_pool.tile([128, 512], mybir.dt.float32, tag="rp")
                    nc.tensor.matmul(
                        rp[:, : b - a], rep_sb[:], gidxf_sb[:, a:b],
                        start=True, stop=True,
                    )
                    nc.scalar.copy(gidx_sb[:, a:b], rp[:, : b - a])
                    rep_state[region] = b
            # srel rides the wire as bf16 (values are small integers, exact)
            # and is widened on device: tensor_scalar's scalar operand must
            # be f32.
            srel_bf = const_pool.tile([128, ncol], mybir.dt.bfloat16)
            nc.sync.dma_start(srel_bf[:], srel_d[:])
            srel_sb = const_pool.tile([128, ncol], mybir.dt.float32)
            nc.vector.tensor_copy(srel_sb[:], srel_bf[:])
            cnt1_sb = const_pool.tile([128, NS], mybir.dt.float32)
            nc.sync.dma_start(cnt1_sb[:], cnt1_d[:])

            iota_i = const_pool.tile([128, 128], mybir.dt.int32)
            nc.gpsimd.iota(iota_i[:], pattern=[[1, 128]], base=0, channel_multiplier=0)
            iota_f = const_pool.tile([128, 128], mybir.dt.bfloat16)
            nc.vector.tensor_copy(iota_f[:], iota_i[:])

            ring_l = const_pool.tile([128, RING_L, D], mybir.dt.bfloat16)
            ring_h = const_pool.tile([128, RING_H, D], mybir.dt.bfloat16)

            def emit_call(ring, ring_sz, src, row0, col0):
                """One full 1024-row gather call of the given stream."""
                s0 = (row0 // 128) % ring_sz
                nc.gpsimd.dma_gather(
                    ring[:, s0 : s0 + MAX_GATHER // 128, :], src,
                    gidx_sb[:, col0 + row0 // 16 : col0 + (row0 + MAX_GATHER) // 16],
                    num_idxs=MAX_GATHER, num_idxs_reg=MAX_GATHER,
                    elem_size=D, elem_step=D,
                )
                return row0 + MAX_GATHER

            done_l = 0  # stream rows gathered so far
            done_h = 0
            col = 0     # srel column cursor (host layout matches this order)
            for g in range(NS):
                # replicate the idx cols this slot's gather calls will read,
                # plus one chunk of lookahead (outside any psum matmul chain)
                rep_to("L", _pad_calls(bL[g] * 128) // 16 + 512)
                rep_to("H", rows_l // 16 + _pad_calls(bH[g] * 128) // 16 + 512)
                while done_l < bL[g] * 128:
                    done_l = emit_call(ring_l, RING_L, feat_lo, done_l, 0)
                while done_h < bH[g] * 128:
                    done_h = emit_call(ring_h, RING_H, feat_hi, done_h,
                                       rows_l // 16)

                n_mm = (bL[g] - aL[g]) + (bH[g] - aH[g])
                psum = psum_pool.tile([128, D], mybir.dt.float32, tag="ps")
                k = 0
                for m in range(aL[g], bL[g]):
                    oh = oh_pool.tile([128, 128], mybir.dt.bfloat16, tag="oh")
                    nc.vector.tensor_scalar(
                        oh[:], iota_f[:], srel_sb[:, col : col + 1], None,
                        op0=mybir.AluOpType.is_equal,
                    )
                    k += 1
                    nc.tensor.matmul(
                        psum[:], oh[:], ring_l[:, m % RING_L, :],
                        start=(k == 1), stop=(k == n_mm),
                    )
                    col += 1
                for m in range(aH[g], bH[g]):
                    oh = oh_pool.tile([128, 128], mybir.dt.bfloat16, tag="oh")
                    nc.vector.tensor_scalar(
                        oh[:], iota_f[:], srel_sb[:, col : col + 1], None,
                        op0=mybir.AluOpType.is_equal,
                    )
                    k += 1
                    nc.tensor.matmul(
                        psum[:], oh[:], ring_h[:, m % RING_H, :],
                        start=(k == 1), stop=(k == n_mm),
                    )
                    col += 1

                rec = fin_pool.tile([128, 1], mybir.dt.float32, tag="rec")
                nc.vector.reciprocal(rec[:], cnt1_sb[:, g : g + 1])
                o_sb = fin_pool.tile([128, D], mybir.dt.bfloat16, tag="o")
                nc.vector.tensor_scalar_mul(o_sb[:], psum[:], rec[:])
                nc.sync.dma_start(out_d[g * 128 : (g + 1) * 128, :], o_sb[:])

    nc.compile()
    return nc


def _preprocess(features, neighbor_idx, segment_ids):
    """Host-side shard/index metadata construction (integers only)."""
    feat = np.ascontiguousarray(np.asarray(features, dtype=np.float32))
    seg = np.asarray(segment_ids).astype(np.int64)
    nid = np.asarray(neighbor_idx).astype(np.int64)
    n_edges = seg.shape[0]

    bf16 = mybir.dt.np(mybir.dt.bfloat16)
    featb = feat.astype(bf16)
    deg = np.bincount(seg, minlength=N_NODES)

    # two-range node sharding: per-core slices of the low and high dest
    # nodes, edge-balanced within each class
    e_low = int(np.searchsorted(seg, SPLIT))
    lowb = [0]
    for c in range(1, N_CORES):
        lowb.append(int(seg[min(c * e_low // N_CORES, max(e_low - 1, 0))]))
    lowb.append(SPLIT)
    highb = [SPLIT]
    for c in range(1, N_CORES):
        highb.append(
            int(seg[min(e_low + c * (n_edges - e_low) // N_CORES, n_edges - 1)])
        )
    highb.append(N_NODES)

    # per-core merged (regular + self-loop) edge lists in dest order, and
    # per-node class-split prefix sums; dest ids are core-relative with the
    # high slice appended after the low slice
    cores = []
    for c in range(N_CORES):
        nn_a = lowb[c + 1] - lowb[c]
        segs, xs = [], []
        for b0, b1, off in (
            (lowb[c], lowb[c + 1], 0),
            (highb[c], highb[c + 1], nn_a),
        ):
            lo, hi = np.searchsorted(seg, [b0, b1])
            nn = b1 - b0
            segs.append(
                np.concatenate([seg[lo:hi] - np.int64(b0), np.arange(nn)]) + off
            )
            xs.append(np.concatenate([nid[lo:hi], np.arange(b0, b1)]))
        s = np.concatenate(segs)
        x = np.concatenate(xs)
        order = np.argsort(s, kind="stable")
        s, x = s[order], x[order]
        nn = nn_a + (highb[c + 1] - highb[c])
        is_l = x < SPLIT
        cum_l = np.concatenate([[0], np.cumsum(np.bincount(s[is_l], minlength=nn))])
        cum_h = np.concatenate([[0], np.cumsum(np.bincount(s[~is_l], minlength=nn))])
        cores.append((s, x, nn_a, nn, cum_l, cum_h))

    # anchored slot packing: per core, choose <=128-node slot boundaries
    # tracking shared cumulative L/H stream targets so every core's slot-g
    # stream interval lands in (nearly) the same tiles
    node_bnds_all = []
    st_l = np.zeros((N_CORES, NS), np.int64)
    en_l = np.zeros((N_CORES, NS), np.int64)
    st_h = np.zeros((N_CORES, NS), np.int64)
    en_h = np.zeros((N_CORES, NS), np.int64)
    for c, (s, x, nn_a, nn, cum_l, cum_h) in enumerate(cores):
        node_bnds = [0]
        i = 0
        for g in range(NS):
            if g == NSA - 1:
                j = nn_a
            elif g == NS - 1:
                j = nn
            else:
                l_mid, h_mid = cum_l[nn_a], cum_h[nn_a]
                if g < NSA:
                    t_l = l_mid * (g + 1) / NSA
                    t_h = h_mid * (g + 1) / NSA
                    part_end = nn_a
                else:
                    t_l = l_mid + (cum_l[nn] - l_mid) * (g + 1 - NSA) / NSB
                    t_h = h_mid + (cum_h[nn] - h_mid) * (g + 1 - NSA) / NSB
                    part_end = nn
                js = np.arange(i + 1, min(i + 128, part_end) + 1)
                cost = np.abs(cum_l[js] - t_l) + np.abs(cum_h[js] - t_h)
                j = int(js[np.argmin(cost)])
            assert j - i <= 128
            st_l[c, g], en_l[c, g] = cum_l[i], cum_l[j]
            st_h[c, g], en_h[c, g] = cum_h[i], cum_h[j]
            node_bnds.append(j)
            i = j
        node_bnds_all.append(node_bnds)

    aL = (st_l.min(0) // 128).tolist()
    bL = (-(-en_l.max(0) // 128)).tolist()
    aH = (st_h.min(0) // 128).tolist()
    bH = (-(-en_h.max(0) // 128)).tolist()
    spans = (tuple(aL), tuple(bL), tuple(aH), tuple(bH))
    nt_l, nt_h = max(bL), max(bH)
    rows_l, rows_h = _pad_calls(nt_l * 128), _pad_calls(nt_h * 128)
    ncol = sum(b - a for a, b in zip(aL, bL)) + sum(
        b - a for a, b in zip(aH, bH)
    )

    in_maps = []
    slot_maps = []
    for c, (s, x, nn_a, nn, cum_l, cum_h) in enumerate(cores):
        node_bnds = node_bnds_all[c]
        is_l = x < SPLIT
        # dense class streams in dest order; within each slot's run, sort
        # by source row for HBM locality (order within a slot is free)
        xl, sl_ = x[is_l], s[is_l]
        xh, sh_ = x[~is_l] - SPLIT, s[~is_l]
        for g in range(NS):
            i, j = node_bnds[g], node_bnds[g + 1]
            for xs_, ss_, cum in ((xl, sl_, cum_l), (xh, sh_, cum_h)):
                a, b = int(cum[i]), int(cum[j])
                o = np.argsort(xs_[a:b], kind="stable")
                xs_[a:b], ss_[a:b] = xs_[a:b][o], ss_[a:b][o]

        gidx_all = np.zeros(rows_l + rows_h, np.int16)
        gidx_all[: len(xl)] = xl.astype(np.int16)
        gidx_all[rows_l : rows_l + len(xh)] = xh.astype(np.int16)

        srel_all = np.full((ncol, 128), -1.0, np.float32)
        cnt1 = np.ones((128, NS), np.float32)
        col = 0
        for g in range(NS):
            i, j = node_bnds[g], node_bnds[g + 1]
            for (a_t, b_t, st, en, ss_) in (
                (aL[g], bL[g], int(cum_l[i]), int(cum_l[j]), sl_),
                (aH[g], bH[g], int(cum_h[i]), int(cum_h[j]), sh_),
            ):
                for m in range(a_t, b_t):
                    r0, r1 = max(128 * m, st), min(128 * m + 128, en)
                    if r1 > r0:
                        srel_all[col, r0 - 128 * m : r1 - 128 * m] = (
                            ss_[r0:r1] - i
                        )
                    col += 1
            width = j - i
            if width:
                if i < nn_a:
                    abs_base = lowb[c] + i
                else:
                    abs_base = highb[c] + (i - nn_a)
                cnt1[:width, g] = 1.0 + deg[abs_base : abs_base + width]
        assert col == ncol

        gidx_w = np.ascontiguousarray(
            gidx_all.reshape(-1, 16).T.astype(np.float32)
        )
        in_maps.append(
            {
                "featb": featb,
                "gidxf": gidx_w,
                "repmat": np.ascontiguousarray(
                    (np.arange(128)[None, :] % 16 == np.arange(16)[:, None])
                    .astype(np.float32)
                ),
                "srel": np.ascontiguousarray(srel_all.T).astype(bf16),
                "cnt1": cnt1,
            }
        )
        sm = []
        for g in range(NS):
            i, j = node_bnds[g], node_bnds[g + 1]
            if i < nn_a:
                sm.append((lowb[c] + i, j - i))
            else:
                sm.append((highb[c] + (i - nn_a), j - i))
        slot_maps.append(sm)
    return spans, in_maps, slot_maps


def kernel(features, neighbor_idx, segment_ids):
    global LAST_NC
    spans, in_maps, slot_maps = _preprocess(
        features, neighbor_idx, segment_ids
    )

    if spans not in _PROGRAM_CACHE:
        _PROGRAM_CACHE[spans] = _build_program(spans)
    nc = _PROGRAM_CACHE[spans]
    LAST_NC = nc

    try:
        res = bass_utils.run_bass_kernel_spmd(
            nc, in_maps, core_ids=list(range(N_CORES))
        )
    except Exception:
        # transient axon/device hiccups (e.g. recovering from a prior wedge)
        # have been observed to clear after a short pause
        import time

        time.sleep(20)
        res = bass_utils.run_bass_kernel_spmd(
            nc, in_maps, core_ids=list(range(N_CORES))
        )

    out = np.empty((N_NODES, D), np.float32)
    for c in range(N_CORES):
        oc = res.results[c]["out"].astype(np.float32)
        for g, (abs_base, width) in enumerate(slot_maps[c]):
            if width:
                out[abs_base : abs_base + width] = oc[g * 128 : g * 128 + width]
    return out


# revision 29
# speedup vs baseline: 1.0032x; 1.0032x over previous
"""GCNAggregator Trainium2 Bass kernel.

out[i] = (sum_{e: seg[e]==i} features[neighbor_idx[e]] + features[i]) / (deg_i + 1)

Strategy (8 NeuronCores, SPMD):
  - dma_gather indices are int16 (max 32767) but the table has 50000 rows,
    so every gathered row is classed L (row < 32768, gathered from the
    table base) or H (row >= 32768, gathered from an offset view). Each
    core is given a contiguous slice of the LOW dest nodes and a
    contiguous slice of the HIGH dest nodes, edge-balanced within each
    class, so per-core L/H gather totals match across cores to ~0.1%
    (one self-loop edge per node is folded in, and self rows are all-L on
    low nodes / all-H on high nodes).
  - Features ride as a bf16 table (512B/row -- the cost model's DMA
    sweet spot). Rel err ~3e-3 end-to-end, well inside the 2e-2 gate.
  - Per core, dest nodes are packed into 51 "slots" of <=128 consecutive
    nodes. The L rows of all slots form one dense stream in dest order
    (H likewise): no per-slot alignment padding. Slot boundaries are
    anchored to shared cumulative targets so every core's slot-g stream
    interval lands within a tile or two of the same place.
  - Streams are gathered with full 1024-descriptor dma_gather calls
    (the SWDGE ring caps at 1024 descriptors per call; bigger rings wedge
    the device) into circular SBUF rings of 128-row tiles, ~1us fixed
    Pool-engine cost per call. Gather calls ignore slot boundaries.
  - Segment-sum per slot on the tensor engine: for each stream tile
    overlapping the slot's interval on ANY core,
        psum[128 nodes, 256] += onehot[128 rows, 128 nodes]^T @ ring[tile]
    with the bf16 one-hot built on device by is_equal(iota, srel) from
    per-(tile,slot) relative dest ids; rows of a boundary tile that
    belong to the neighboring slot carry srel -1 there and are picked up
    by that slot's own matmul over the same tile. PSUM accumulates fp32.
  - Finalize per slot: out = psum * 1/(deg+1) (bf16), DMA out.
  - Gather indices ride the wire once as [16, ni16] f32 and are
    replicated on device to the 128-partition wrapped layout the SWDGE
    ucode requires (8 gpsimd cores read their own partition group) via
    one-hot f32 matmuls + Activation-engine PSUM->int16 copies, produced
    just-in-time between slot chains.
  - Engine budget per core (TimelineSim): DMA ~309us (97% busy, the
    bottleneck: 208k gather descriptors x 512B at 360GB/s aggregate),
    Pool ~273us, PE ~235us, DVE ~203us, Act ~17us.

The host only computes integer index metadata (shard boundaries, stream
index layouts, relative segment ids, degrees); all floating point work
(gather, segment sum, normalize) runs on device.
"""

import os as _os
import sys

import numpy as np

try:
    import concourse  # noqa: F401
except ImportError:  # pragma: no cover
    sys.path.insert(0, "/opt/trn_rl_repo")

from contextlib import ExitStack

import concourse.mybir as mybir
from concourse import bacc, bass_utils, tile

N_NODES = 50000
N_EDGES = 1_600_000
D = 256
N_CORES = 8
SPLIT = 32768       # int16 gather-index window
NSA = 33            # slots covering the core's low-node slice
NSB = 18            # slots covering the core's high-node slice
NS = NSA + NSB

_PROGRAM_CACHE: dict = {}
LAST_NC = None  # exposed for test harness introspection (TimelineSim)

MAX_GATHER = 1024   # SWDGE descriptor ring capacity per dma_gather call
RING_L = 96         # L-stream SBUF ring, in 128-row tiles (multiple of 8)
RING_H = 56         # H-stream ring


def _pad_calls(rows):
    return -(-rows // MAX_GATHER) * MAX_GATHER


def _build_program(spans):
    """Build + compile the (uniform-across-cores, SPMD) per-core program.

    spans = (aL, bL, aH, bH): per-slot stream-tile intervals, the union
    over the 8 cores of each slot's L/H stream coverage. The program
    matmuls every (slot, tile) pair in these intervals; per-core srel
    data masks which rows of the tile actually belong to the slot.
    """
    aL, bL, aH, bH, rows_l_act, rows_h_act = (
        list(spans[0]), list(spans[1]), list(spans[2]), list(spans[3]),
        spans[4], spans[5],
    )
    nt_l, nt_h = max(bL), max(bH)
    rows_l, rows_h = _pad_calls(nt_l * 128), _pad_calls(nt_h * 128)
    ni16 = (rows_l + rows_h) // 16
    ncol = sum(b - a for a, b in zip(aL, bL)) + sum(
        b - a for a, b in zip(aH, bH)
    )

    nc = bacc.Bacc(
        "TRN2", target_bir_lowering=False, debug=False, num_devices=N_CORES,
    )

    feat_d = nc.dram_tensor(
        "featb", (N_NODES, D), mybir.dt.bfloat16, kind="ExternalInput"
    ).ap()
    gidx_d = nc.dram_tensor(
        "gidxi", (16, ni16), mybir.dt.int16, kind="ExternalInput"
    ).ap()
    rep_d = nc.dram_tensor(
        "repmat", (16, 128), mybir.dt.float32, kind="ExternalInput"
    ).ap()
    srel_d = nc.dram_tensor(
        "srel", (128, ncol), mybir.dt.bfloat16, kind="ExternalInput"
    ).ap()
    cnt1_d = nc.dram_tensor(
        "cnt1", (128, NS), mybir.dt.float32, kind="ExternalInput"
    ).ap()
    out_d = nc.dram_tensor(
        "out", (NS * 128, D), mybir.dt.bfloat16, kind="ExternalOutput"
    ).ap()

    feat_lo = feat_d[0:SPLIT, :]
    feat_hi = feat_d[SPLIT:N_NODES, :]

    with tile.TileContext(nc) as tc:
        with ExitStack() as ctx:
            ob = int(_os.environ.get("OH_BUFS", "4"))
            fb = int(_os.environ.get("FIN_BUFS", "3"))
            pb = int(_os.environ.get("PSUM_BUFS", "4"))
            const_pool = ctx.enter_context(tc.tile_pool(name="const", bufs=1))
            oh_pool = ctx.enter_context(tc.tile_pool(name="oh", bufs=ob))
            fin_pool = ctx.enter_context(tc.tile_pool(name="fin", bufs=fb))
            psum_pool = ctx.enter_context(
                tc.tile_pool(name="psum", bufs=pb, space="PSUM")
            )

            # gather indices ride the wire once as [16, ni16] f32 (exact for
            # idx < 2^24) and are replicated to the 128-partition wrapped
            # layout the SWDGE ucode needs via one-hot f32 matmuls on the
            # (otherwise idle-at-start) tensor engine, with PSUM->SBUF int16
            # copies on the idle Activation engine. Chunks are produced
            # just-in-time between slot accumulation chains, ahead of the
            # gather calls that read them.
            gidx_sb = const_pool.tile([128, ni16], mybir.dt.int16)
            gidxi_sb = const_pool.tile([16, ni16], mybir.dt.int16)
            gidxf_sb = const_pool.tile([16, ni16], mybir.dt.float32)
            rep_sb = const_pool.tile([16, 128], mybir.dt.float32)
            nc.sync.dma_start(rep_sb[:], rep_d[:])
            n_ld = 8
            ld_bounds = [ni16 * i // n_ld for i in range(n_ld + 1)]
            for a, b in zip(ld_bounds[:-1], ld_bounds[1:]):
                if b > a:
                    nc.sync.dma_start(gidxi_sb[:, a:b], gidx_d[:, a:b])
            rep_pool = ctx.enter_context(
                tc.tile_pool(name="rpsum", bufs=2, space="PSUM")
            )
            # independent replication cursors for the L and H col regions
            rep_state = {"L": 0, "H": rows_l // 16}
            rep_end = {"L": rows_l // 16, "H": ni16}

            def rep_to(region, col_need):
                while rep_state[region] < min(col_need, rep_end[region]):
                    a = rep_state[region]
                    b = min(a + 512, rep_end[region])
                    nc.vector.tensor_copy(gidxf_sb[:, a:b], gidxi_sb[:, a:b])
                    rp = rep_pool.tile([128, 512], mybir.dt.float32, tag="rp")
                    nc.tensor.matmul(
                        rp[:, : b - a], rep_sb[:], gidxf_sb[:, a:b],
                        start=True, stop=True,
                    )
                    nc.scalar.copy(gidx_sb[:, a:b], rp[:, : b - a])
                    rep_state[region] = b
            # srel rides the wire as bf16 (values are small integers, exact)
            # and is widened on device: tensor_scalar's scalar operand must
            # be f32.
            srel_bf = const_pool.tile([128, ncol], mybir.dt.bfloat16)
            nc.sync.dma_start(srel_bf[:], srel_d[:])
            srel_sb = const_pool.tile([128, ncol], mybir.dt.float32)
            nc.vector.tensor_copy(srel_sb[:], srel_bf[:])
            cnt1_sb = const_pool.tile([128, NS], mybir.dt.float32)
            nc.sync.dma_start(cnt1_sb[:], cnt1_d[:])

            iota_i = const_pool.tile([128, 128], mybir.dt.int32)
            nc.gpsimd.iota(iota_i[:], pattern=[[1, 128]], base=0, channel_multiplier=0)
            iota_f = const_pool.tile([128, 128], mybir.dt.bfloat16)
            nc.vector.tensor_copy(iota_f[:], iota_i[:])

            ring_l = const_pool.tile([128, RING_L, D], mybir.dt.bfloat16)
            ring_h = const_pool.tile([128, RING_H, D], mybir.dt.bfloat16)

            def emit_call(ring, ring_sz, src, row0, col0, rows_end):
                """One <=1024-row gather call of the given stream."""
                k = min(MAX_GATHER, rows_end - row0)
                s0 = (row0 // 128) % ring_sz
                nc.gpsimd.dma_gather(
                    ring[:, s0 : s0 + -(-k // 128), :], src,
                    gidx_sb[:, col0 + row0 // 16 : col0 + (row0 + k) // 16],
                    num_idxs=k, num_idxs_reg=k,
                    elem_size=D, elem_step=D,
                )
                return row0 + k

            done_l = 0  # stream rows gathered so far
            done_h = 0
            col = 0     # srel column cursor (host layout matches this order)
            for g in range(NS):
                # replicate the idx cols this slot's gather calls will read,
                # plus one chunk of lookahead (outside any psum matmul chain)
                rep_to("L", _pad_calls(bL[g] * 128) // 16 + 512)
                rep_to("H", rows_l // 16 + _pad_calls(bH[g] * 128) // 16 + 512)
                while done_l < min(bL[g] * 128, rows_l_act):
                    done_l = emit_call(ring_l, RING_L, feat_lo, done_l, 0,
                                       rows_l_act)
                while done_h < min(bH[g] * 128, rows_h_act):
                    done_h = emit_call(ring_h, RING_H, feat_hi, done_h,
                                       rows_l // 16, rows_h_act)

                n_mm = (bL[g] - aL[g]) + (bH[g] - aH[g])
                psum = psum_pool.tile([128, D], mybir.dt.float32, tag="ps")
                k = 0
                for m in range(aL[g], bL[g]):
                    oh = oh_pool.tile([128, 128], mybir.dt.bfloat16, tag="oh")
                    nc.vector.tensor_scalar(
                        oh[:], iota_f[:], srel_sb[:, col : col + 1], None,
                        op0=mybir.AluOpType.is_equal,
                    )
                    k += 1
                    nc.tensor.matmul(
                        psum[:], oh[:], ring_l[:, m % RING_L, :],
                        start=(k == 1), stop=(k == n_mm),
                    )
                    col += 1
                for m in range(aH[g], bH[g]):
                    oh = oh_pool.tile([128, 128], mybir.dt.bfloat16, tag="oh")
                    nc.vector.tensor_scalar(
                        oh[:], iota_f[:], srel_sb[:, col : col + 1], None,
                        op0=mybir.AluOpType.is_equal,
                    )
                    k += 1
                    nc.tensor.matmul(
                        psum[:], oh[:], ring_h[:, m % RING_H, :],
                        start=(k == 1), stop=(k == n_mm),
                    )
                    col += 1

                rec = fin_pool.tile([128, 1], mybir.dt.float32, tag="rec")
                nc.vector.reciprocal(rec[:], cnt1_sb[:, g : g + 1])
                o_sb = fin_pool.tile([128, D], mybir.dt.bfloat16, tag="o")
                nc.vector.tensor_scalar_mul(o_sb[:], psum[:], rec[:])
                nc.sync.dma_start(out_d[g * 128 : (g + 1) * 128, :], o_sb[:])

    nc.compile()
    return nc


def _preprocess(features, neighbor_idx, segment_ids):
    """Host-side shard/index metadata construction (integers only)."""
    feat = np.ascontiguousarray(np.asarray(features, dtype=np.float32))
    seg = np.asarray(segment_ids).astype(np.int64)
    nid = np.asarray(neighbor_idx).astype(np.int64)
    n_edges = seg.shape[0]

    bf16 = mybir.dt.np(mybir.dt.bfloat16)
    featb = feat.astype(bf16)
    deg = np.bincount(seg, minlength=N_NODES)

    # two-range node sharding: per-core slices of the low and high dest
    # nodes, edge-balanced within each class
    e_low = int(np.searchsorted(seg, SPLIT))
    lowb = [0]
    for c in range(1, N_CORES):
        lowb.append(int(seg[min(c * e_low // N_CORES, max(e_low - 1, 0))]))
    lowb.append(SPLIT)
    highb = [SPLIT]
    for c in range(1, N_CORES):
        highb.append(
            int(seg[min(e_low + c * (n_edges - e_low) // N_CORES, n_edges - 1)])
        )
    highb.append(N_NODES)

    # per-core merged (regular + self-loop) edge lists in dest order, and
    # per-node class-split prefix sums; dest ids are core-relative with the
    # high slice appended after the low slice
    cores = []
    for c in range(N_CORES):
        nn_a = lowb[c + 1] - lowb[c]
        segs, xs = [], []
        for b0, b1, off in (
            (lowb[c], lowb[c + 1], 0),
            (highb[c], highb[c + 1], nn_a),
        ):
            lo, hi = np.searchsorted(seg, [b0, b1])
            nn = b1 - b0
            segs.append(
                np.concatenate([seg[lo:hi] - np.int64(b0), np.arange(nn)]) + off
            )
            xs.append(np.concatenate([nid[lo:hi], np.arange(b0, b1)]))
        s = np.concatenate(segs)
        x = np.concatenate(xs)
        order = np.argsort(s, kind="stable")
        s, x = s[order], x[order]
        nn = nn_a + (highb[c + 1] - highb[c])
        is_l = x < SPLIT
        cum_l = np.concatenate([[0], np.cumsum(np.bincount(s[is_l], minlength=nn))])
        cum_h = np.concatenate([[0], np.cumsum(np.bincount(s[~is_l], minlength=nn))])
        cores.append((s, x, nn_a, nn, cum_l, cum_h))

    # anchored slot packing: per core, choose <=128-node slot boundaries
    # tracking shared cumulative L/H stream targets so every core's slot-g
    # stream interval lands in (nearly) the same tiles
    node_bnds_all = []
    st_l = np.zeros((N_CORES, NS), np.int64)
    en_l = np.zeros((N_CORES, NS), np.int64)
    st_h = np.zeros((N_CORES, NS), np.int64)
    en_h = np.zeros((N_CORES, NS), np.int64)
    for c, (s, x, nn_a, nn, cum_l, cum_h) in enumerate(cores):
        node_bnds = [0]
        i = 0
        for g in range(NS):
            if g == NSA - 1:
                j = nn_a
            elif g == NS - 1:
                j = nn
            else:
                l_mid, h_mid = cum_l[nn_a], cum_h[nn_a]
                if g < NSA:
                    t_l = l_mid * (g + 1) / NSA
                    t_h = h_mid * (g + 1) / NSA
                    part_end = nn_a
                else:
                    t_l = l_mid + (cum_l[nn] - l_mid) * (g + 1 - NSA) / NSB
                    t_h = h_mid + (cum_h[nn] - h_mid) * (g + 1 - NSA) / NSB
                    part_end = nn
                js = np.arange(i + 1, min(i + 128, part_end) + 1)
                cost = np.abs(cum_l[js] - t_l) + np.abs(cum_h[js] - t_h)
                j = int(js[np.argmin(cost)])
            assert j - i <= 128
            st_l[c, g], en_l[c, g] = cum_l[i], cum_l[j]
            st_h[c, g], en_h[c, g] = cum_h[i], cum_h[j]
            node_bnds.append(j)
            i = j
        node_bnds_all.append(node_bnds)

    aL = (st_l.min(0) // 128).tolist()
    bL = (-(-en_l.max(0) // 128)).tolist()
    aH = (st_h.min(0) // 128).tolist()
    bH = (-(-en_h.max(0) // 128)).tolist()
    rows_l_act = -(-int(en_l.max(0)[-1]) // 16) * 16
    rows_h_act = -(-int(en_h.max(0)[-1]) // 16) * 16
    spans = (tuple(aL), tuple(bL), tuple(aH), tuple(bH), rows_l_act, rows_h_act)
    nt_l, nt_h = max(bL), max(bH)
    rows_l, rows_h = _pad_calls(nt_l * 128), _pad_calls(nt_h * 128)
    ncol = sum(b - a for a, b in zip(aL, bL)) + sum(
        b - a for a, b in zip(aH, bH)
    )

    in_maps = []
    slot_maps = []
    for c, (s, x, nn_a, nn, cum_l, cum_h) in enumerate(cores):
        node_bnds = node_bnds_all[c]
        is_l = x < SPLIT
        # dense class streams in dest order; within each slot's run, sort
        # by source row for HBM locality (order within a slot is free)
        xl, sl_ = x[is_l], s[is_l]
        xh, sh_ = x[~is_l] - SPLIT, s[~is_l]
        for g in range(NS):
            i, j = node_bnds[g], node_bnds[g + 1]
            for xs_, ss_, cum in ((xl, sl_, cum_l), (xh, sh_, cum_h)):
                a, b = int(cum[i]), int(cum[j])
                o = np.argsort(xs_[a:b], kind="stable")
                xs_[a:b], ss_[a:b] = xs_[a:b][o], ss_[a:b][o]

        gidx_all = np.zeros(rows_l + rows_h, np.int16)
        gidx_all[: len(xl)] = xl.astype(np.int16)
        gidx_all[rows_l : rows_l + len(xh)] = xh.astype(np.int16)

        srel_all = np.full((ncol, 128), -1.0, np.float32)
        cnt1 = np.ones((128, NS), np.float32)
        col = 0
        for g in range(NS):
            i, j = node_bnds[g], node_bnds[g + 1]
            for (a_t, b_t, st, en, ss_) in (
                (aL[g], bL[g], int(cum_l[i]), int(cum_l[j]), sl_),
                (aH[g], bH[g], int(cum_h[i]), int(cum_h[j]), sh_),
            ):
                for m in range(a_t, b_t):
                    r0, r1 = max(128 * m, st), min(128 * m + 128, en)
                    if r1 > r0:
                        srel_all[col, r0 - 128 * m : r1 - 128 * m] = (
                            ss_[r0:r1] - i
                        )
                    col += 1
            width = j - i
            if width:
                if i < nn_a:
                    abs_base = lowb[c] + i
                else:
                    abs_base = highb[c] + (i - nn_a)
                cnt1[:width, g] = 1.0 + deg[abs_base : abs_base + width]
        assert col == ncol

        gidx_w = np.ascontiguousarray(gidx_all.reshape(-1, 16).T)
        in_maps.append(
            {
                "featb": featb,
                "gidxi": gidx_w,
                "repmat": np.ascontiguousarray(
                    (np.arange(128)[None, :] % 16 == np.arange(16)[:, None])
                    .astype(np.float32)
                ),
                "srel": np.ascontiguousarray(srel_all.T).astype(bf16),
                "cnt1": cnt1,
            }
        )
        sm = []
        for g in range(NS):
            i, j = node_bnds[g], node_bnds[g + 1]
            if i < nn_a:
                sm.append((lowb[c] + i, j - i))
            else:
                sm.append((highb[c] + (i - nn_a), j - i))
        slot_maps.append(sm)
    return spans, in_maps, slot_maps


def kernel(features, neighbor_idx, segment_ids):
    global LAST_NC
    spans, in_maps, slot_maps = _preprocess(
        features, neighbor_idx, segment_ids
    )

    if spans not in _PROGRAM_CACHE:
        _PROGRAM_CACHE[spans] = _build_program(spans)
    nc = _PROGRAM_CACHE[spans]
    LAST_NC = nc

    try:
        res = bass_utils.run_bass_kernel_spmd(
            nc, in_maps, core_ids=list(range(N_CORES))
        )
    except Exception:
        # transient axon/device hiccups (e.g. recovering from a prior wedge)
        # have been observed to clear after a short pause
        import time

        time.sleep(20)
        res = bass_utils.run_bass_kernel_spmd(
            nc, in_maps, core_ids=list(range(N_CORES))
        )

    out = np.empty((N_NODES, D), np.float32)
    for c in range(N_CORES):
        oc = res.results[c]["out"].astype(np.float32)
        for g, (abs_base, width) in enumerate(slot_maps[c]):
            if width:
                out[abs_base : abs_base + width] = oc[g * 128 : g * 128 + width]
    return out


# revision 30
# speedup vs baseline: 1.0046x; 1.0014x over previous
"""GCNAggregator Trainium2 Bass kernel.

out[i] = (sum_{e: seg[e]==i} features[neighbor_idx[e]] + features[i]) / (deg_i + 1)

Strategy (8 NeuronCores, SPMD):
  - dma_gather indices are int16 (max 32767) but the table has 50000 rows,
    so every gathered row is classed L (row < 32768, gathered from the
    table base) or H (row >= 32768, gathered from an offset view). Each
    core is given a contiguous slice of the LOW dest nodes and a
    contiguous slice of the HIGH dest nodes, edge-balanced within each
    class, so per-core L/H gather totals match across cores to ~0.1%
    (one self-loop edge per node is folded in, and self rows are all-L on
    low nodes / all-H on high nodes).
  - Features ride as a bf16 table (512B/row -- the cost model's DMA
    sweet spot). Rel err ~3e-3 end-to-end, well inside the 2e-2 gate.
  - Per core, dest nodes are packed into 51 "slots" of <=128 consecutive
    nodes. The L rows of all slots form one dense stream in dest order
    (H likewise): no per-slot alignment padding. Slot boundaries are
    anchored to shared cumulative targets so every core's slot-g stream
    interval lands within a tile or two of the same place.
  - Streams are gathered with full 1024-descriptor dma_gather calls
    (the SWDGE ring caps at 1024 descriptors per call; bigger rings wedge
    the device) into circular SBUF rings of 128-row tiles, ~1us fixed
    Pool-engine cost per call. Gather calls ignore slot boundaries.
  - Segment-sum per slot on the tensor engine: for each stream tile
    overlapping the slot's interval on ANY core,
        psum[128 nodes, 256] += onehot[128 rows, 128 nodes]^T @ ring[tile]
    with the bf16 one-hot built on device by is_equal(iota, srel) from
    per-(tile,slot) relative dest ids; rows of a boundary tile that
    belong to the neighboring slot carry srel -1 there and are picked up
    by that slot's own matmul over the same tile. PSUM accumulates fp32.
  - Finalize per slot: out = psum * 1/(deg+1) (bf16), DMA out.
  - Gather indices ride the wire once as [16, ni16] int16 and are
    replicated on device to the 128-partition wrapped layout the SWDGE
    ucode requires (8 gpsimd cores read their own partition group) via
    one-hot f32 matmuls + Activation-engine PSUM->int16 copies, produced
    just-in-time between slot chains.
  - Engine budget per core (TimelineSim): DMA ~306us (96% busy, the
    bottleneck: 206.4k gather descriptors x 512B at 360GB/s aggregate,
    within 0.1% of the zero-padding floor), Pool ~272us, PE ~237us,
    DVE ~212us, Act ~17us.

The host only computes integer index metadata (shard boundaries, stream
index layouts, relative segment ids, degrees); all floating point work
(gather, segment sum, normalize) runs on device.
"""

import os as _os
import sys

import numpy as np

try:
    import concourse  # noqa: F401
except ImportError:  # pragma: no cover
    sys.path.insert(0, "/opt/trn_rl_repo")

from contextlib import ExitStack

import concourse.mybir as mybir
from concourse import bacc, bass_utils, tile

N_NODES = 50000
N_EDGES = 1_600_000
D = 256
N_CORES = 8
SPLIT = 32768       # int16 gather-index window
NSA = 33            # slots covering the core's low-node slice
NSB = 18            # slots covering the core's high-node slice
NS = NSA + NSB

_PROGRAM_CACHE: dict = {}
LAST_NC = None  # exposed for test harness introspection (TimelineSim)

MAX_GATHER = 1024   # SWDGE descriptor ring capacity per dma_gather call
RING_L = 96         # L-stream SBUF ring, in 128-row tiles (multiple of 8)
RING_H = 56         # H-stream ring


def _pad_calls(rows):
    return -(-rows // MAX_GATHER) * MAX_GATHER


def _build_program(spans):
    """Build + compile the (uniform-across-cores, SPMD) per-core program.

    spans = (aL, bL, aH, bH): per-slot stream-tile intervals, the union
    over the 8 cores of each slot's L/H stream coverage. The program
    matmuls every (slot, tile) pair in these intervals; per-core srel
    data masks which rows of the tile actually belong to the slot.
    """
    aL, bL, aH, bH, rows_l_act, rows_h_act = (
        list(spans[0]), list(spans[1]), list(spans[2]), list(spans[3]),
        spans[4], spans[5],
    )
    nt_l, nt_h = max(bL), max(bH)
    rows_l, rows_h = _pad_calls(nt_l * 128), _pad_calls(nt_h * 128)
    ni16 = (rows_l + rows_h) // 16
    ncol = sum(b - a for a, b in zip(aL, bL)) + sum(
        b - a for a, b in zip(aH, bH)
    )

    nc = bacc.Bacc(
        "TRN2", target_bir_lowering=False, debug=False, num_devices=N_CORES,
    )

    feat_d = nc.dram_tensor(
        "featb", (N_NODES, D), mybir.dt.bfloat16, kind="ExternalInput"
    ).ap()
    gidx_d = nc.dram_tensor(
        "gidxi", (16, ni16), mybir.dt.int16, kind="ExternalInput"
    ).ap()
    rep_d = nc.dram_tensor(
        "repmat", (16, 128), mybir.dt.float32, kind="ExternalInput"
    ).ap()
    srel_d = nc.dram_tensor(
        "srel", (128, ncol), mybir.dt.bfloat16, kind="ExternalInput"
    ).ap()
    cnt1_d = nc.dram_tensor(
        "cnt1", (128, NS), mybir.dt.float32, kind="ExternalInput"
    ).ap()
    out_d = nc.dram_tensor(
        "out", (NS * 128, D), mybir.dt.bfloat16, kind="ExternalOutput"
    ).ap()

    feat_lo = feat_d[0:SPLIT, :]
    feat_hi = feat_d[SPLIT:N_NODES, :]

    with tile.TileContext(nc) as tc:
        with ExitStack() as ctx:
            ob = int(_os.environ.get("OH_BUFS", "6"))
            fb = int(_os.environ.get("FIN_BUFS", "3"))
            pb = int(_os.environ.get("PSUM_BUFS", "4"))
            const_pool = ctx.enter_context(tc.tile_pool(name="const", bufs=1))
            oh_pool = ctx.enter_context(tc.tile_pool(name="oh", bufs=ob))
            fin_pool = ctx.enter_context(tc.tile_pool(name="fin", bufs=fb))
            psum_pool = ctx.enter_context(
                tc.tile_pool(name="psum", bufs=pb, space="PSUM")
            )

            # gather indices ride the wire once as [16, ni16] f32 (exact for
            # idx < 2^24) and are replicated to the 128-partition wrapped
            # layout the SWDGE ucode needs via one-hot f32 matmuls on the
            # (otherwise idle-at-start) tensor engine, with PSUM->SBUF int16
            # copies on the idle Activation engine. Chunks are produced
            # just-in-time between slot accumulation chains, ahead of the
            # gather calls that read them.
            gidx_sb = const_pool.tile([128, ni16], mybir.dt.int16)
            gidxi_sb = const_pool.tile([16, ni16], mybir.dt.int16)
            gidxf_sb = const_pool.tile([16, ni16], mybir.dt.float32)
            rep_sb = const_pool.tile([16, 128], mybir.dt.float32)
            nc.sync.dma_start(rep_sb[:], rep_d[:])
            n_ld = 8
            ld_bounds = [ni16 * i // n_ld for i in range(n_ld + 1)]
            for a, b in zip(ld_bounds[:-1], ld_bounds[1:]):
                if b > a:
                    nc.sync.dma_start(gidxi_sb[:, a:b], gidx_d[:, a:b])
            rep_pool = ctx.enter_context(
                tc.tile_pool(name="rpsum", bufs=2, space="PSUM")
            )
            # independent replication cursors for the L and H col regions
            rep_state = {"L": 0, "H": rows_l // 16}
            rep_end = {"L": rows_l // 16, "H": ni16}

            def rep_to(region, col_need):
                while rep_state[region] < min(col_need, rep_end[region]):
                    a = rep_state[region]
                    b = min(a + 512, rep_end[region])
                    nc.vector.tensor_copy(gidxf_sb[:, a:b], gidxi_sb[:, a:b])
                    rp = rep_pool.tile([128, 512], mybir.dt.float32, tag="rp")
                    nc.tensor.matmul(
                        rp[:, : b - a], rep_sb[:], gidxf_sb[:, a:b],
                        start=True, stop=True,
                    )
                    nc.scalar.copy(gidx_sb[:, a:b], rp[:, : b - a])
                    rep_state[region] = b
            # srel rides the wire as bf16 (values are small integers, exact)
            # and is widened on device: tensor_scalar's scalar operand must
            # be f32.
            srel_bf = const_pool.tile([128, ncol], mybir.dt.bfloat16)
            nc.sync.dma_start(srel_bf[:], srel_d[:])
            srel_sb = const_pool.tile([128, ncol], mybir.dt.float32)
            nc.vector.tensor_copy(srel_sb[:], srel_bf[:])
            cnt1_sb = const_pool.tile([128, NS], mybir.dt.float32)
            nc.sync.dma_start(cnt1_sb[:], cnt1_d[:])

            iota_i = const_pool.tile([128, 128], mybir.dt.int32)
            nc.gpsimd.iota(iota_i[:], pattern=[[1, 128]], base=0, channel_multiplier=0)
            iota_f = const_pool.tile([128, 128], mybir.dt.bfloat16)
            nc.vector.tensor_copy(iota_f[:], iota_i[:])

            ring_l = const_pool.tile([128, RING_L, D], mybir.dt.bfloat16)
            ring_h = const_pool.tile([128, RING_H, D], mybir.dt.bfloat16)

            def emit_call(ring, ring_sz, src, row0, col0, rows_end):
                """One <=1024-row gather call of the given stream."""
                k = min(MAX_GATHER, rows_end - row0)
                s0 = (row0 // 128) % ring_sz
                nc.gpsimd.dma_gather(
                    ring[:, s0 : s0 + -(-k // 128), :], src,
                    gidx_sb[:, col0 + row0 // 16 : col0 + (row0 + k) // 16],
                    num_idxs=k, num_idxs_reg=k,
                    elem_size=D, elem_step=D,
                )
                return row0 + k

            done_l = 0  # stream rows gathered so far
            done_h = 0
            col = 0     # srel column cursor (host layout matches this order)
            for g in range(NS):
                # replicate the idx cols this slot's gather calls will read,
                # plus one chunk of lookahead (outside any psum matmul chain)
                rep_to("L", _pad_calls(bL[g] * 128) // 16 + 512)
                rep_to("H", rows_l // 16 + _pad_calls(bH[g] * 128) // 16 + 512)
                while done_l < min(bL[g] * 128, rows_l_act):
                    done_l = emit_call(ring_l, RING_L, feat_lo, done_l, 0,
                                       rows_l_act)
                while done_h < min(bH[g] * 128, rows_h_act):
                    done_h = emit_call(ring_h, RING_H, feat_hi, done_h,
                                       rows_l // 16, rows_h_act)

                n_mm = (bL[g] - aL[g]) + (bH[g] - aH[g])
                psum = psum_pool.tile([128, D], mybir.dt.float32, tag="ps")
                k = 0
                for m in range(aL[g], bL[g]):
                    oh = oh_pool.tile([128, 128], mybir.dt.bfloat16, tag="oh")
                    nc.vector.tensor_scalar(
                        oh[:], iota_f[:], srel_sb[:, col : col + 1], None,
                        op0=mybir.AluOpType.is_equal,
                    )
                    k += 1
                    nc.tensor.matmul(
                        psum[:], oh[:], ring_l[:, m % RING_L, :],
                        start=(k == 1), stop=(k == n_mm),
                    )
                    col += 1
                for m in range(aH[g], bH[g]):
                    oh = oh_pool.tile([128, 128], mybir.dt.bfloat16, tag="oh")
                    nc.vector.tensor_scalar(
                        oh[:], iota_f[:], srel_sb[:, col : col + 1], None,
                        op0=mybir.AluOpType.is_equal,
                    )
                    k += 1
                    nc.tensor.matmul(
                        psum[:], oh[:], ring_h[:, m % RING_H, :],
                        start=(k == 1), stop=(k == n_mm),
                    )
                    col += 1

                rec = fin_pool.tile([128, 1], mybir.dt.float32, tag="rec")
                nc.vector.reciprocal(rec[:], cnt1_sb[:, g : g + 1])
                o_sb = fin_pool.tile([128, D], mybir.dt.bfloat16, tag="o")
                nc.vector.tensor_scalar_mul(o_sb[:], psum[:], rec[:])
                nc.sync.dma_start(out_d[g * 128 : (g + 1) * 128, :], o_sb[:])

    nc.compile()
    return nc


def _preprocess(features, neighbor_idx, segment_ids):
    """Host-side shard/index metadata construction (integers only)."""
    feat = np.ascontiguousarray(np.asarray(features, dtype=np.float32))
    seg = np.asarray(segment_ids).astype(np.int64)
    nid = np.asarray(neighbor_idx).astype(np.int64)
    n_edges = seg.shape[0]

    bf16 = mybir.dt.np(mybir.dt.bfloat16)
    featb = feat.astype(bf16)
    deg = np.bincount(seg, minlength=N_NODES)

    # two-range node sharding: per-core slices of the low and high dest
    # nodes, edge-balanced within each class
    e_low = int(np.searchsorted(seg, SPLIT))
    lowb = [0]
    for c in range(1, N_CORES):
        lowb.append(int(seg[min(c * e_low // N_CORES, max(e_low - 1, 0))]))
    lowb.append(SPLIT)
    highb = [SPLIT]
    for c in range(1, N_CORES):
        highb.append(
            int(seg[min(e_low + c * (n_edges - e_low) // N_CORES, n_edges - 1)])
        )
    highb.append(N_NODES)

    # per-core merged (regular + self-loop) edge lists in dest order, and
    # per-node class-split prefix sums; dest ids are core-relative with the
    # high slice appended after the low slice
    cores = []
    for c in range(N_CORES):
        nn_a = lowb[c + 1] - lowb[c]
        segs, xs = [], []
        for b0, b1, off in (
            (lowb[c], lowb[c + 1], 0),
            (highb[c], highb[c + 1], nn_a),
        ):
            lo, hi = np.searchsorted(seg, [b0, b1])
            nn = b1 - b0
            segs.append(
                np.concatenate([seg[lo:hi] - np.int64(b0), np.arange(nn)]) + off
            )
            xs.append(np.concatenate([nid[lo:hi], np.arange(b0, b1)]))
        s = np.concatenate(segs)
        x = np.concatenate(xs)
        order = np.argsort(s, kind="stable")
        s, x = s[order], x[order]
        nn = nn_a + (highb[c + 1] - highb[c])
        is_l = x < SPLIT
        cum_l = np.concatenate([[0], np.cumsum(np.bincount(s[is_l], minlength=nn))])
        cum_h = np.concatenate([[0], np.cumsum(np.bincount(s[~is_l], minlength=nn))])
        cores.append((s, x, nn_a, nn, cum_l, cum_h))

    # anchored slot packing: per core, choose <=128-node slot boundaries
    # tracking shared cumulative L/H stream targets so every core's slot-g
    # stream interval lands in (nearly) the same tiles
    node_bnds_all = []
    st_l = np.zeros((N_CORES, NS), np.int64)
    en_l = np.zeros((N_CORES, NS), np.int64)
    st_h = np.zeros((N_CORES, NS), np.int64)
    en_h = np.zeros((N_CORES, NS), np.int64)
    for c, (s, x, nn_a, nn, cum_l, cum_h) in enumerate(cores):
        node_bnds = [0]
        i = 0
        for g in range(NS):
            if g == NSA - 1:
                j = nn_a
            elif g == NS - 1:
                j = nn
            else:
                l_mid, h_mid = cum_l[nn_a], cum_h[nn_a]
                if g < NSA:
                    t_l = l_mid * (g + 1) / NSA
                    t_h = h_mid * (g + 1) / NSA
                    part_end = nn_a
                else:
                    t_l = l_mid + (cum_l[nn] - l_mid) * (g + 1 - NSA) / NSB
                    t_h = h_mid + (cum_h[nn] - h_mid) * (g + 1 - NSA) / NSB
                    part_end = nn
                js = np.arange(i + 1, min(i + 128, part_end) + 1)
                cost = np.abs(cum_l[js] - t_l) + np.abs(cum_h[js] - t_h)
                j = int(js[np.argmin(cost)])
            assert j - i <= 128
            st_l[c, g], en_l[c, g] = cum_l[i], cum_l[j]
            st_h[c, g], en_h[c, g] = cum_h[i], cum_h[j]
            node_bnds.append(j)
            i = j
        node_bnds_all.append(node_bnds)

    aL = (st_l.min(0) // 128).tolist()
    bL = (-(-en_l.max(0) // 128)).tolist()
    aH = (st_h.min(0) // 128).tolist()
    bH = (-(-en_h.max(0) // 128)).tolist()
    rows_l_act = -(-int(en_l.max(0)[-1]) // 16) * 16
    rows_h_act = -(-int(en_h.max(0)[-1]) // 16) * 16
    spans = (tuple(aL), tuple(bL), tuple(aH), tuple(bH), rows_l_act, rows_h_act)
    nt_l, nt_h = max(bL), max(bH)
    rows_l, rows_h = _pad_calls(nt_l * 128), _pad_calls(nt_h * 128)
    ncol = sum(b - a for a, b in zip(aL, bL)) + sum(
        b - a for a, b in zip(aH, bH)
    )

    in_maps = []
    slot_maps = []
    for c, (s, x, nn_a, nn, cum_l, cum_h) in enumerate(cores):
        node_bnds = node_bnds_all[c]
        is_l = x < SPLIT
        # dense class streams in dest order; within each slot's run, sort
        # by source row for HBM locality (order within a slot is free)
        xl, sl_ = x[is_l], s[is_l]
        xh, sh_ = x[~is_l] - SPLIT, s[~is_l]
        for g in range(NS):
            i, j = node_bnds[g], node_bnds[g + 1]
            for xs_, ss_, cum in ((xl, sl_, cum_l), (xh, sh_, cum_h)):
                a, b = int(cum[i]), int(cum[j])
                o = np.argsort(xs_[a:b], kind="stable")
                xs_[a:b], ss_[a:b] = xs_[a:b][o], ss_[a:b][o]

        gidx_all = np.zeros(rows_l + rows_h, np.int16)
        gidx_all[: len(xl)] = xl.astype(np.int16)
        gidx_all[rows_l : rows_l + len(xh)] = xh.astype(np.int16)

        srel_all = np.full((ncol, 128), -1.0, np.float32)
        cnt1 = np.ones((128, NS), np.float32)
        col = 0
        for g in range(NS):
            i, j = node_bnds[g], node_bnds[g + 1]
            for (a_t, b_t, st, en, ss_) in (
                (aL[g], bL[g], int(cum_l[i]), int(cum_l[j]), sl_),
                (aH[g], bH[g], int(cum_h[i]), int(cum_h[j]), sh_),
            ):
                for m in range(a_t, b_t):
                    r0, r1 = max(128 * m, st), min(128 * m + 128, en)
                    if r1 > r0:
                        srel_all[col, r0 - 128 * m : r1 - 128 * m] = (
                            ss_[r0:r1] - i
                        )
                    col += 1
            width = j - i
            if width:
                if i < nn_a:
                    abs_base = lowb[c] + i
                else:
                    abs_base = highb[c] + (i - nn_a)
                cnt1[:width, g] = 1.0 + deg[abs_base : abs_base + width]
        assert col == ncol

        gidx_w = np.ascontiguousarray(gidx_all.reshape(-1, 16).T)
        in_maps.append(
            {
                "featb": featb,
                "gidxi": gidx_w,
                "repmat": np.ascontiguousarray(
                    (np.arange(128)[None, :] % 16 == np.arange(16)[:, None])
                    .astype(np.float32)
                ),
                "srel": np.ascontiguousarray(srel_all.T).astype(bf16),
                "cnt1": cnt1,
            }
        )
        sm = []
        for g in range(NS):
            i, j = node_bnds[g], node_bnds[g + 1]
            if i < nn_a:
                sm.append((lowb[c] + i, j - i))
            else:
                sm.append((highb[c] + (i - nn_a), j - i))
        slot_maps.append(sm)
    return spans, in_maps, slot_maps


def kernel(features, neighbor_idx, segment_ids):
    global LAST_NC
    spans, in_maps, slot_maps = _preprocess(
        features, neighbor_idx, segment_ids
    )

    if spans not in _PROGRAM_CACHE:
        _PROGRAM_CACHE[spans] = _build_program(spans)
    nc = _PROGRAM_CACHE[spans]
    LAST_NC = nc

    try:
        res = bass_utils.run_bass_kernel_spmd(
            nc, in_maps, core_ids=list(range(N_CORES))
        )
    except Exception:
        # transient axon/device hiccups (e.g. recovering from a prior wedge)
        # have been observed to clear after a short pause
        import time

        time.sleep(20)
        res = bass_utils.run_bass_kernel_spmd(
            nc, in_maps, core_ids=list(range(N_CORES))
        )

    out = np.empty((N_NODES, D), np.float32)
    for c in range(N_CORES):
        oc = res.results[c]["out"].astype(np.float32)
        for g, (abs_base, width) in enumerate(slot_maps[c]):
            if width:
                out[abs_base : abs_base + width] = oc[g * 128 : g * 128 + width]
    return out


# revision 35
# speedup vs baseline: 1.0066x; 1.0020x over previous
"""GCNAggregator Trainium2 Bass kernel.

out[i] = (sum_{e: seg[e]==i} features[neighbor_idx[e]] + features[i]) / (deg_i + 1)

Strategy (8 NeuronCores, SPMD):
  - dma_gather indices are int16 (max 32767) but the table has 50000 rows,
    so every gathered row is classed L (row < 32768, gathered from the
    table base) or H (row >= 32768, gathered from an offset view). Each
    core is given a contiguous slice of the LOW dest nodes and a
    contiguous slice of the HIGH dest nodes, edge-balanced within each
    class, so per-core L/H gather totals match across cores to ~0.1%
    (one self-loop edge per node is folded in, and self rows are all-L on
    low nodes / all-H on high nodes).
  - Features ride as a bf16 table (512B/row -- the cost model's DMA
    sweet spot). Rel err ~3e-3 end-to-end, well inside the 2e-2 gate.
  - Per core, dest nodes are packed into 51 "slots" of <=128 consecutive
    nodes. The L rows of all slots form one dense stream in dest order
    (H likewise): no per-slot alignment padding. Slot boundaries are
    anchored to shared cumulative targets so every core's slot-g stream
    interval lands within a tile or two of the same place.
  - Streams are gathered with full 1024-descriptor dma_gather calls
    (the SWDGE ring caps at 1024 descriptors per call; bigger rings wedge
    the device) into circular SBUF rings of 128-row tiles, ~1us fixed
    Pool-engine cost per call. Gather calls ignore slot boundaries.
  - Segment-sum per slot on the tensor engine: for each stream tile
    overlapping the slot's interval on ANY core,
        psum[128 nodes, 256] += onehot[128 rows, 128 nodes]^T @ ring[tile]
    with the bf16 one-hot built on device by is_equal(iota, srel) from
    per-(tile,slot) relative dest ids; rows of a boundary tile that
    belong to the neighboring slot carry srel -1 there and are picked up
    by that slot's own matmul over the same tile. PSUM accumulates fp32.
  - Finalize per slot: out = psum * 1/(deg+1) (bf16), DMA out.
  - Gather indices ride the wire once as [16, ni16] int16 and are
    replicated on device to the 128-partition wrapped layout the SWDGE
    ucode requires (8 gpsimd cores read their own partition group) via
    one-hot f32 matmuls + Activation-engine PSUM->int16 copies, produced
    just-in-time between slot chains.
  - Engine budget per core (TimelineSim): DMA ~306us (96% busy, the
    bottleneck: 206.4k gather descriptors x 512B at 360GB/s aggregate,
    within 0.1% of the zero-padding floor), Pool ~272us, PE ~237us,
    DVE ~212us, Act ~17us.

The host only computes integer index metadata (shard boundaries, stream
index layouts, relative segment ids, degrees); all floating point work
(gather, segment sum, normalize) runs on device.
"""

import os as _os
import sys

import numpy as np

try:
    import concourse  # noqa: F401
except ImportError:  # pragma: no cover
    sys.path.insert(0, "/opt/trn_rl_repo")

from contextlib import ExitStack

import concourse.mybir as mybir
from concourse import bacc, bass_utils, tile

N_NODES = 50000
N_EDGES = 1_600_000
D = 256
N_CORES = 8
SPLIT = 32768       # int16 gather-index window
NSA = 33            # slots covering the core's low-node slice
NSB = 18            # slots covering the core's high-node slice
NS = NSA + NSB

_PROGRAM_CACHE: dict = {}
LAST_NC = None  # exposed for test harness introspection (TimelineSim)

MAX_GATHER = 1024   # SWDGE descriptor ring capacity per dma_gather call
RING_L = 96         # L-stream SBUF ring, in 128-row tiles (multiple of 8)
RING_H = 56         # H-stream ring


def _pad_calls(rows):
    return -(-rows // MAX_GATHER) * MAX_GATHER


def _build_program(spans):
    """Build + compile the (uniform-across-cores, SPMD) per-core program.

    spans = (aL, bL, aH, bH): per-slot stream-tile intervals, the union
    over the 8 cores of each slot's L/H stream coverage. The program
    matmuls every (slot, tile) pair in these intervals; per-core srel
    data masks which rows of the tile actually belong to the slot.
    """
    aL, bL, aH, bH, rows_l_act, rows_h_act = (
        list(spans[0]), list(spans[1]), list(spans[2]), list(spans[3]),
        spans[4], spans[5],
    )
    nt_l, nt_h = max(bL), max(bH)
    rows_l, rows_h = _pad_calls(nt_l * 128), _pad_calls(nt_h * 128)
    ni16 = (rows_l + rows_h) // 16
    ncol = sum(b - a for a, b in zip(aL, bL)) + sum(
        b - a for a, b in zip(aH, bH)
    )

    nc = bacc.Bacc(
        "TRN2", target_bir_lowering=False, debug=False, num_devices=N_CORES,
    )

    feat_d = nc.dram_tensor(
        "featb", (N_NODES, D), mybir.dt.bfloat16, kind="ExternalInput"
    ).ap()
    gidx_d = nc.dram_tensor(
        "gidxi", (16, ni16), mybir.dt.int16, kind="ExternalInput"
    ).ap()
    rep_d = nc.dram_tensor(
        "repmat", (16, 128), mybir.dt.float32, kind="ExternalInput"
    ).ap()
    srel_d = nc.dram_tensor(
        "srel", (128, ncol), mybir.dt.uint8, kind="ExternalInput"
    ).ap()
    cnt1_d = nc.dram_tensor(
        "cnt1", (128, NS), mybir.dt.float32, kind="ExternalInput"
    ).ap()
    out_d = nc.dram_tensor(
        "out", (NS * 128, D), mybir.dt.bfloat16, kind="ExternalOutput"
    ).ap()

    feat_lo = feat_d[0:SPLIT, :]
    feat_hi = feat_d[SPLIT:N_NODES, :]

    with tile.TileContext(nc) as tc:
        with ExitStack() as ctx:
            ob = int(_os.environ.get("OH_BUFS", "6"))
            fb = int(_os.environ.get("FIN_BUFS", "3"))
            pb = int(_os.environ.get("PSUM_BUFS", "4"))
            const_pool = ctx.enter_context(tc.tile_pool(name="const", bufs=1))
            oh_pool = ctx.enter_context(tc.tile_pool(name="oh", bufs=ob))
            fin_pool = ctx.enter_context(tc.tile_pool(name="fin", bufs=fb))
            psum_pool = ctx.enter_context(
                tc.tile_pool(name="psum", bufs=pb, space="PSUM")
            )

            # gather indices ride the wire once as [16, ni16] f32 (exact for
            # idx < 2^24) and are replicated to the 128-partition wrapped
            # layout the SWDGE ucode needs via one-hot f32 matmuls on the
            # (otherwise idle-at-start) tensor engine, with PSUM->SBUF int16
            # copies on the idle Activation engine. Chunks are produced
            # just-in-time between slot accumulation chains, ahead of the
            # gather calls that read them.
            gidx_sb = const_pool.tile([128, ni16], mybir.dt.int16)
            gidxi_sb = const_pool.tile([16, ni16], mybir.dt.int16)
            gidxf_sb = const_pool.tile([16, ni16], mybir.dt.float32)
            rep_sb = const_pool.tile([16, 128], mybir.dt.float32)
            nc.sync.dma_start(rep_sb[:], rep_d[:])
            n_ld = 8
            ld_bounds = [ni16 * i // n_ld for i in range(n_ld + 1)]
            for a, b in zip(ld_bounds[:-1], ld_bounds[1:]):
                if b > a:
                    nc.sync.dma_start(gidxi_sb[:, a:b], gidx_d[:, a:b])
            rep_pool = ctx.enter_context(
                tc.tile_pool(name="rpsum", bufs=2, space="PSUM")
            )
            # independent replication cursors for the L and H col regions
            rep_state = {"L": 0, "H": rows_l // 16}
            rep_end = {"L": rows_l // 16, "H": ni16}

            def rep_to(region, col_need):
                while rep_state[region] < min(col_need, rep_end[region]):
                    a = rep_state[region]
                    b = min(a + 512, rep_end[region])
                    nc.vector.tensor_copy(gidxf_sb[:, a:b], gidxi_sb[:, a:b])
                    rp = rep_pool.tile([128, 512], mybir.dt.float32, tag="rp")
                    nc.tensor.matmul(
                        rp[:, : b - a], rep_sb[:], gidxf_sb[:, a:b],
                        start=True, stop=True,
                    )
                    nc.scalar.copy(gidx_sb[:, a:b], rp[:, : b - a])
                    rep_state[region] = b
            # srel rides the wire as bf16 (values are small integers, exact)
            # and is widened on device: tensor_scalar's scalar operand must
            # be f32.
            srel_bf = const_pool.tile([128, ncol], mybir.dt.uint8)
            nc.sync.dma_start(srel_bf[:], srel_d[:])
            srel_sb = const_pool.tile([128, ncol], mybir.dt.float32)
            nc.vector.tensor_copy(srel_sb[:], srel_bf[:])
            cnt1_sb = const_pool.tile([128, NS], mybir.dt.float32)
            nc.sync.dma_start(cnt1_sb[:], cnt1_d[:])

            iota_i = const_pool.tile([128, 128], mybir.dt.int32)
            nc.gpsimd.iota(iota_i[:], pattern=[[1, 128]], base=0, channel_multiplier=0)
            iota_f = const_pool.tile([128, 128], mybir.dt.bfloat16)
            nc.vector.tensor_copy(iota_f[:], iota_i[:])

            ring_l = const_pool.tile([128, RING_L, D], mybir.dt.bfloat16)
            ring_h = const_pool.tile([128, RING_H, D], mybir.dt.bfloat16)

            def emit_call(ring, ring_sz, src, row0, col0, rows_end):
                """One <=1024-row gather call of the given stream."""
                k = min(MAX_GATHER, rows_end - row0)
                s0 = (row0 // 128) % ring_sz
                nc.gpsimd.dma_gather(
                    ring[:, s0 : s0 + -(-k // 128), :], src,
                    gidx_sb[:, col0 + row0 // 16 : col0 + (row0 + k) // 16],
                    num_idxs=k, num_idxs_reg=k,
                    elem_size=D, elem_step=D,
                )
                return row0 + k

            done_l = 0  # stream rows gathered so far
            done_h = 0
            col = 0     # srel column cursor (host layout matches this order)
            for g in range(NS):
                # replicate the idx cols this slot's gather calls will read,
                # plus one chunk of lookahead (outside any psum matmul chain)
                rep_to("L", _pad_calls(bL[g] * 128) // 16 + 512)
                rep_to("H", rows_l // 16 + _pad_calls(bH[g] * 128) // 16 + 512)
                while done_l < min(bL[g] * 128, rows_l_act):
                    done_l = emit_call(ring_l, RING_L, feat_lo, done_l, 0,
                                       rows_l_act)
                while done_h < min(bH[g] * 128, rows_h_act):
                    done_h = emit_call(ring_h, RING_H, feat_hi, done_h,
                                       rows_l // 16, rows_h_act)

                n_mm = (bL[g] - aL[g]) + (bH[g] - aH[g])
                psum = psum_pool.tile([128, D], mybir.dt.float32, tag="ps")
                k = 0
                for m in range(aL[g], bL[g]):
                    oh = oh_pool.tile([128, 128], mybir.dt.bfloat16, tag="oh")
                    nc.vector.tensor_scalar(
                        oh[:], iota_f[:], srel_sb[:, col : col + 1], None,
                        op0=mybir.AluOpType.is_equal,
                    )
                    k += 1
                    nc.tensor.matmul(
                        psum[:], oh[:], ring_l[:, m % RING_L, :],
                        start=(k == 1), stop=(k == n_mm),
                    )
                    col += 1
                for m in range(aH[g], bH[g]):
                    oh = oh_pool.tile([128, 128], mybir.dt.bfloat16, tag="oh")
                    nc.vector.tensor_scalar(
                        oh[:], iota_f[:], srel_sb[:, col : col + 1], None,
                        op0=mybir.AluOpType.is_equal,
                    )
                    k += 1
                    nc.tensor.matmul(
                        psum[:], oh[:], ring_h[:, m % RING_H, :],
                        start=(k == 1), stop=(k == n_mm),
                    )
                    col += 1

                rec = fin_pool.tile([128, 1], mybir.dt.float32, tag="rec")
                nc.vector.reciprocal(rec[:], cnt1_sb[:, g : g + 1])
                o_sb = fin_pool.tile([128, D], mybir.dt.bfloat16, tag="o")
                nc.vector.tensor_scalar_mul(o_sb[:], psum[:], rec[:])
                nc.sync.dma_start(out_d[g * 128 : (g + 1) * 128, :], o_sb[:])

    nc.compile()
    return nc


def _preprocess(features, neighbor_idx, segment_ids):
    """Host-side shard/index metadata construction (integers only)."""
    feat = np.ascontiguousarray(np.asarray(features, dtype=np.float32))
    seg = np.asarray(segment_ids).astype(np.int64)
    nid = np.asarray(neighbor_idx).astype(np.int64)
    n_edges = seg.shape[0]

    bf16 = mybir.dt.np(mybir.dt.bfloat16)
    featb = feat.astype(bf16)
    deg = np.bincount(seg, minlength=N_NODES)

    # two-range node sharding: per-core slices of the low and high dest
    # nodes, edge-balanced within each class
    e_low = int(np.searchsorted(seg, SPLIT))
    lowb = [0]
    for c in range(1, N_CORES):
        lowb.append(int(seg[min(c * e_low // N_CORES, max(e_low - 1, 0))]))
    lowb.append(SPLIT)
    highb = [SPLIT]
    for c in range(1, N_CORES):
        highb.append(
            int(seg[min(e_low + c * (n_edges - e_low) // N_CORES, n_edges - 1)])
        )
    highb.append(N_NODES)

    # per-core merged (regular + self-loop) edge lists in dest order, and
    # per-node class-split prefix sums; dest ids are core-relative with the
    # high slice appended after the low slice
    cores = []
    for c in range(N_CORES):
        nn_a = lowb[c + 1] - lowb[c]
        segs, xs = [], []
        for b0, b1, off in (
            (lowb[c], lowb[c + 1], 0),
            (highb[c], highb[c + 1], nn_a),
        ):
            lo, hi = np.searchsorted(seg, [b0, b1])
            nn = b1 - b0
            segs.append(
                np.concatenate([seg[lo:hi] - np.int64(b0), np.arange(nn)]) + off
            )
            xs.append(np.concatenate([nid[lo:hi], np.arange(b0, b1)]))
        s = np.concatenate(segs)
        x = np.concatenate(xs)
        order = np.argsort(s, kind="stable")
        s, x = s[order], x[order]
        nn = nn_a + (highb[c + 1] - highb[c])
        is_l = x < SPLIT
        cum_l = np.concatenate([[0], np.cumsum(np.bincount(s[is_l], minlength=nn))])
        cum_h = np.concatenate([[0], np.cumsum(np.bincount(s[~is_l], minlength=nn))])
        cores.append((s, x, nn_a, nn, cum_l, cum_h))

    # anchored slot packing: per core, choose <=128-node slot boundaries
    # tracking shared cumulative L/H stream targets so every core's slot-g
    # stream interval lands in (nearly) the same tiles
    node_bnds_all = []
    st_l = np.zeros((N_CORES, NS), np.int64)
    en_l = np.zeros((N_CORES, NS), np.int64)
    st_h = np.zeros((N_CORES, NS), np.int64)
    en_h = np.zeros((N_CORES, NS), np.int64)
    for c, (s, x, nn_a, nn, cum_l, cum_h) in enumerate(cores):
        node_bnds = [0]
        i = 0
        for g in range(NS):
            if g == NSA - 1:
                j = nn_a
            elif g == NS - 1:
                j = nn
            else:
                l_mid, h_mid = cum_l[nn_a], cum_h[nn_a]
                if g < NSA:
                    t_l = l_mid * (g + 1) / NSA
                    t_h = h_mid * (g + 1) / NSA
                    part_end = nn_a
                else:
                    t_l = l_mid + (cum_l[nn] - l_mid) * (g + 1 - NSA) / NSB
                    t_h = h_mid + (cum_h[nn] - h_mid) * (g + 1 - NSA) / NSB
                    part_end = nn
                js = np.arange(i + 1, min(i + 128, part_end) + 1)
                cost = np.abs(cum_l[js] - t_l) + np.abs(cum_h[js] - t_h)
                j = int(js[np.argmin(cost)])
            assert j - i <= 128
            st_l[c, g], en_l[c, g] = cum_l[i], cum_l[j]
            st_h[c, g], en_h[c, g] = cum_h[i], cum_h[j]
            node_bnds.append(j)
            i = j
        node_bnds_all.append(node_bnds)

    aL = (st_l.min(0) // 128).tolist()
    bL = (-(-en_l.max(0) // 128)).tolist()
    aH = (st_h.min(0) // 128).tolist()
    bH = (-(-en_h.max(0) // 128)).tolist()
    rows_l_act = -(-int(en_l.max(0)[-1]) // 16) * 16
    rows_h_act = -(-int(en_h.max(0)[-1]) // 16) * 16
    spans = (tuple(aL), tuple(bL), tuple(aH), tuple(bH), rows_l_act, rows_h_act)
    nt_l, nt_h = max(bL), max(bH)
    rows_l, rows_h = _pad_calls(nt_l * 128), _pad_calls(nt_h * 128)
    ncol = sum(b - a for a, b in zip(aL, bL)) + sum(
        b - a for a, b in zip(aH, bH)
    )

    in_maps = []
    slot_maps = []
    for c, (s, x, nn_a, nn, cum_l, cum_h) in enumerate(cores):
        node_bnds = node_bnds_all[c]
        is_l = x < SPLIT
        # dense class streams in dest order; within each slot's run, sort
        # by source row for HBM locality (order within a slot is free)
        xl, sl_ = x[is_l], s[is_l]
        xh, sh_ = x[~is_l] - SPLIT, s[~is_l]
        for g in range(NS):
            i, j = node_bnds[g], node_bnds[g + 1]
            for xs_, ss_, cum in ((xl, sl_, cum_l), (xh, sh_, cum_h)):
                a, b = int(cum[i]), int(cum[j])
                o = np.argsort(xs_[a:b], kind="stable")
                xs_[a:b], ss_[a:b] = xs_[a:b][o], ss_[a:b][o]

        gidx_all = np.zeros(rows_l + rows_h, np.int16)
        gidx_all[: len(xl)] = xl.astype(np.int16)
        gidx_all[rows_l : rows_l + len(xh)] = xh.astype(np.int16)

        srel_all = np.full((ncol, 128), -1.0, np.float32)
        cnt1 = np.ones((128, NS), np.float32)
        col = 0
        for g in range(NS):
            i, j = node_bnds[g], node_bnds[g + 1]
            for (a_t, b_t, st, en, ss_) in (
                (aL[g], bL[g], int(cum_l[i]), int(cum_l[j]), sl_),
                (aH[g], bH[g], int(cum_h[i]), int(cum_h[j]), sh_),
            ):
                for m in range(a_t, b_t):
                    r0, r1 = max(128 * m, st), min(128 * m + 128, en)
                    if r1 > r0:
                        srel_all[col, r0 - 128 * m : r1 - 128 * m] = (
                            ss_[r0:r1] - i
                        )
                    col += 1
            width = j - i
            if width:
                if i < nn_a:
                    abs_base = lowb[c] + i
                else:
                    abs_base = highb[c] + (i - nn_a)
                cnt1[:width, g] = 1.0 + deg[abs_base : abs_base + width]
        assert col == ncol

        gidx_w = np.ascontiguousarray(gidx_all.reshape(-1, 16).T)
        in_maps.append(
            {
                "featb": featb,
                "gidxi": gidx_w,
                "repmat": np.ascontiguousarray(
                    (np.arange(128)[None, :] % 16 == np.arange(16)[:, None])
                    .astype(np.float32)
                ),
                "srel": np.ascontiguousarray(
                    np.where(srel_all.T < 0, 255.0, srel_all.T)
                ).astype(np.uint8),
                "cnt1": cnt1,
            }
        )
        sm = []
        for g in range(NS):
            i, j = node_bnds[g], node_bnds[g + 1]
            if i < nn_a:
                sm.append((lowb[c] + i, j - i))
            else:
                sm.append((highb[c] + (i - nn_a), j - i))
        slot_maps.append(sm)
    return spans, in_maps, slot_maps


def kernel(features, neighbor_idx, segment_ids):
    global LAST_NC
    spans, in_maps, slot_maps = _preprocess(
        features, neighbor_idx, segment_ids
    )

    if spans not in _PROGRAM_CACHE:
        _PROGRAM_CACHE[spans] = _build_program(spans)
    nc = _PROGRAM_CACHE[spans]
    LAST_NC = nc

    try:
        res = bass_utils.run_bass_kernel_spmd(
            nc, in_maps, core_ids=list(range(N_CORES))
        )
    except Exception:
        # transient axon/device hiccups (e.g. recovering from a prior wedge)
        # have been observed to clear after a short pause
        import time

        time.sleep(20)
        res = bass_utils.run_bass_kernel_spmd(
            nc, in_maps, core_ids=list(range(N_CORES))
        )

    out = np.empty((N_NODES, D), np.float32)
    for c in range(N_CORES):
        oc = res.results[c]["out"].astype(np.float32)
        for g, (abs_base, width) in enumerate(slot_maps[c]):
            if width:
                out[abs_base : abs_base + width] = oc[g * 128 : g * 128 + width]
    return out


# revision 40
# speedup vs baseline: 1.0098x; 1.0032x over previous
"""GCNAggregator Trainium2 Bass kernel.

out[i] = (sum_{e: seg[e]==i} features[neighbor_idx[e]] + features[i]) / (deg_i + 1)

Strategy (8 NeuronCores, SPMD):
  - dma_gather indices are int16 (max 32767) but the table has 50000 rows,
    so every gathered row is classed L (row < 32768, gathered from the
    table base) or H (row >= 32768, gathered from an offset view). Each
    core is given a contiguous slice of the LOW dest nodes and a
    contiguous slice of the HIGH dest nodes, edge-balanced within each
    class, so per-core L/H gather totals match across cores to ~0.1%
    (one self-loop edge per node is folded in, and self rows are all-L on
    low nodes / all-H on high nodes).
  - Features ride as a bf16 table (512B/row -- the cost model's DMA
    sweet spot). Rel err ~3e-3 end-to-end, well inside the 2e-2 gate.
  - Per core, dest nodes are packed into 51 "slots" of <=128 consecutive
    nodes. The L rows of all slots form one dense stream in dest order
    (H likewise): no per-slot alignment padding. Slot boundaries are
    anchored to shared cumulative targets so every core's slot-g stream
    interval lands within a tile or two of the same place.
  - Streams are gathered with full 1024-descriptor dma_gather calls
    (the SWDGE ring caps at 1024 descriptors per call; bigger rings wedge
    the device) into circular SBUF rings of 128-row tiles, ~1us fixed
    Pool-engine cost per call. Gather calls ignore slot boundaries.
  - Segment-sum per slot on the tensor engine: for each stream tile
    overlapping the slot's interval on ANY core,
        psum[128 nodes, 256] += onehot[128 rows, 128 nodes]^T @ ring[tile]
    with the bf16 one-hot built on device by is_equal(iota, srel) from
    per-(tile,slot) relative dest ids; rows of a boundary tile that
    belong to the neighboring slot carry srel -1 there and are picked up
    by that slot's own matmul over the same tile. PSUM accumulates fp32.
  - Finalize per slot: out = psum * 1/(deg+1) (bf16), DMA out.
  - Gather indices ride the wire once as [16, ni16] int16 and are
    replicated on device to the 128-partition wrapped layout the SWDGE
    ucode requires (8 gpsimd cores read their own partition group) via
    one-hot f32 matmuls + Activation-engine PSUM->int16 copies, produced
    just-in-time between slot chains.
  - Engine budget per core (TimelineSim): DMA ~306us (96% busy, the
    bottleneck: 206.4k gather descriptors x 512B at 360GB/s aggregate,
    within 0.1% of the zero-padding floor), Pool ~272us, PE ~237us,
    DVE ~212us, Act ~17us.

The host only computes integer index metadata (shard boundaries, stream
index layouts, relative segment ids, degrees); all floating point work
(gather, segment sum, normalize) runs on device.
"""

import os as _os
import sys

import numpy as np

try:
    import concourse  # noqa: F401
except ImportError:  # pragma: no cover
    sys.path.insert(0, "/opt/trn_rl_repo")

from contextlib import ExitStack

import concourse.mybir as mybir
from concourse import bacc, bass_utils, tile

N_NODES = 50000
N_EDGES = 1_600_000
D = 256
N_CORES = 8
SPLIT = 32768       # int16 gather-index window
NSA = 33            # slots covering the core's low-node slice
NSB = 18            # slots covering the core's high-node slice
NS = NSA + NSB

_PROGRAM_CACHE: dict = {}
LAST_NC = None  # exposed for test harness introspection (TimelineSim)

MAX_GATHER = 1024   # SWDGE descriptor ring capacity per dma_gather call
RING_L = 96         # L-stream SBUF ring, in 128-row tiles (multiple of 8)
RING_H = 56         # H-stream ring


def _pad_calls(rows):
    return -(-rows // MAX_GATHER) * MAX_GATHER


def _build_program(spans):
    """Build + compile the (uniform-across-cores, SPMD) per-core program.

    spans = (aL, bL, aH, bH): per-slot stream-tile intervals, the union
    over the 8 cores of each slot's L/H stream coverage. The program
    matmuls every (slot, tile) pair in these intervals; per-core srel
    data masks which rows of the tile actually belong to the slot.
    """
    aL, bL, aH, bH, rows_l_act, rows_h_act = (
        list(spans[0]), list(spans[1]), list(spans[2]), list(spans[3]),
        spans[4], spans[5],
    )
    nt_l, nt_h = max(bL), max(bH)
    rows_l, rows_h = _pad_calls(nt_l * 128), _pad_calls(nt_h * 128)
    ni16 = (rows_l + rows_h) // 16
    ncol = sum(b - a for a, b in zip(aL, bL)) + sum(
        b - a for a, b in zip(aH, bH)
    )

    nc = bacc.Bacc(
        "TRN2", target_bir_lowering=False, debug=False, num_devices=N_CORES,
    )

    feat_d = nc.dram_tensor(
        "featb", (N_NODES, D), mybir.dt.bfloat16, kind="ExternalInput"
    ).ap()
    gidx_d = nc.dram_tensor(
        "gidxi", (16, ni16), mybir.dt.int16, kind="ExternalInput"
    ).ap()
    rep_d = nc.dram_tensor(
        "repmat", (16, 128), mybir.dt.float32, kind="ExternalInput"
    ).ap()
    srel_d = nc.dram_tensor(
        "srel", (128, ncol), mybir.dt.uint8, kind="ExternalInput"
    ).ap()
    cnt1_d = nc.dram_tensor(
        "cnt1", (128, NS), mybir.dt.float32, kind="ExternalInput"
    ).ap()
    out_d = nc.dram_tensor(
        "out", (NS * 128, D), mybir.dt.bfloat16, kind="ExternalOutput"
    ).ap()

    feat_lo = feat_d[0:SPLIT, :]
    feat_hi = feat_d[SPLIT:N_NODES, :]

    with tile.TileContext(nc) as tc:
        with ExitStack() as ctx:
            ob = int(_os.environ.get("OH_BUFS", "6"))
            fb = int(_os.environ.get("FIN_BUFS", "3"))
            pb = int(_os.environ.get("PSUM_BUFS", "4"))
            const_pool = ctx.enter_context(tc.tile_pool(name="const", bufs=1))
            oh_pool = ctx.enter_context(tc.tile_pool(name="oh", bufs=ob))
            fin_pool = ctx.enter_context(tc.tile_pool(name="fin", bufs=fb))
            psum_pool = ctx.enter_context(
                tc.tile_pool(name="psum", bufs=pb, space="PSUM")
            )

            # gather indices ride the wire once as [16, ni16] f32 (exact for
            # idx < 2^24) and are replicated to the 128-partition wrapped
            # layout the SWDGE ucode needs via one-hot f32 matmuls on the
            # (otherwise idle-at-start) tensor engine, with PSUM->SBUF int16
            # copies on the idle Activation engine. Chunks are produced
            # just-in-time between slot accumulation chains, ahead of the
            # gather calls that read them.
            gidx_sb = const_pool.tile([128, ni16], mybir.dt.int16)
            gidxi_sb = const_pool.tile([16, ni16], mybir.dt.int16)
            gidxf_sb = const_pool.tile([16, ni16], mybir.dt.float32)
            rep_sb = const_pool.tile([16, 128], mybir.dt.float32)
            # head-of-stream idx cols first (small), so the widen->replicate
            # ->desc-gen chain for the first gather calls starts ASAP; the
            # HWDGE pipeline issues one copy per ~650ns, so order matters
            h0 = rows_l // 16
            nc.sync.dma_start(gidxi_sb[:, 0:512], gidx_d[:, 0:512])
            nc.sync.dma_start(rep_sb[:], rep_d[:])
            nc.sync.dma_start(gidxi_sb[:, h0 : h0 + 512], gidx_d[:, h0 : h0 + 512])
            for a, b in [(512, h0), (h0 + 512, ni16)]:
                n_ld = 3
                bnds = [a + (b - a) * i // n_ld for i in range(n_ld + 1)]
                for c0, c1 in zip(bnds[:-1], bnds[1:]):
                    if c1 > c0:
                        nc.sync.dma_start(gidxi_sb[:, c0:c1], gidx_d[:, c0:c1])
            rep_pool = ctx.enter_context(
                tc.tile_pool(name="rpsum", bufs=2, space="PSUM")
            )
            # independent replication cursors for the L and H col regions
            rep_state = {"L": 0, "H": rows_l // 16}
            rep_end = {"L": rows_l // 16, "H": ni16}

            rep_start = dict(rep_state)

            def rep_to(region, col_need):
                while rep_state[region] < min(col_need, rep_end[region]):
                    a = rep_state[region]
                    step = 128 if a - rep_start[region] < 512 else 512
                    b = min(a + step, rep_end[region])
                    nc.vector.tensor_copy(gidxf_sb[:, a:b], gidxi_sb[:, a:b])
                    rp = rep_pool.tile([128, 512], mybir.dt.float32, tag="rp")
                    nc.tensor.matmul(
                        rp[:, : b - a], rep_sb[:], gidxf_sb[:, a:b],
                        start=True, stop=True,
                    )
                    nc.scalar.copy(gidx_sb[:, a:b], rp[:, : b - a])
                    rep_state[region] = b
            # srel rides the wire as bf16 (values are small integers, exact)
            # and is widened on device: tensor_scalar's scalar operand must
            # be f32.
            srel_bf = const_pool.tile([128, ncol], mybir.dt.uint8)
            nc.sync.dma_start(srel_bf[:], srel_d[:])
            srel_sb = const_pool.tile([128, ncol], mybir.dt.float32)
            nc.vector.tensor_copy(srel_sb[:], srel_bf[:])
            cnt1_sb = const_pool.tile([128, NS], mybir.dt.float32)
            nc.sync.dma_start(cnt1_sb[:], cnt1_d[:])

            iota_i = const_pool.tile([128, 128], mybir.dt.int32)
            nc.gpsimd.iota(iota_i[:], pattern=[[1, 128]], base=0, channel_multiplier=0)
            iota_f = const_pool.tile([128, 128], mybir.dt.bfloat16)
            nc.vector.tensor_copy(iota_f[:], iota_i[:])

            ring_l = const_pool.tile([128, RING_L, D], mybir.dt.bfloat16)
            ring_h = const_pool.tile([128, RING_H, D], mybir.dt.bfloat16)


            def emit_call(ring, ring_sz, src, row0, col0, rows_end):
                """One <=1024-row gather call of the given stream."""
                k = min(MAX_GATHER, rows_end - row0)
                s0 = (row0 // 128) % ring_sz
                nc.gpsimd.dma_gather(
                    ring[:, s0 : s0 + -(-k // 128), :], src,
                    gidx_sb[:, col0 + row0 // 16 : col0 + (row0 + k) // 16],
                    num_idxs=k, num_idxs_reg=k,
                    elem_size=D, elem_step=D,
                )
                return row0 + k

            done_l = 0  # stream rows gathered so far
            done_h = 0
            col = 0     # srel column cursor (host layout matches this order)
            for g in range(NS):
                # replicate the idx cols this slot's gather calls will read,
                # plus one chunk of lookahead (outside any psum matmul chain)
                rep_to("L", _pad_calls(bL[g] * 128) // 16 + 512)
                rep_to("H", rows_l // 16 + _pad_calls(bH[g] * 128) // 16 + 512)
                while done_l < min(bL[g] * 128, rows_l_act):
                    done_l = emit_call(ring_l, RING_L, feat_lo, done_l, 0,
                                       rows_l_act)
                while done_h < min(bH[g] * 128, rows_h_act):
                    done_h = emit_call(ring_h, RING_H, feat_hi, done_h,
                                       rows_l // 16, rows_h_act)

                n_mm = (bL[g] - aL[g]) + (bH[g] - aH[g])
                psum = psum_pool.tile([128, D], mybir.dt.float32, tag="ps")
                k = 0
                for ring, ring_sz, a_t, b_t in (
                    (ring_l, RING_L, aL[g], bL[g]),
                    (ring_h, RING_H, aH[g], bH[g]),
                ):
                    for m in range(a_t, b_t):
                        oh = oh_pool.tile(
                            [128, 128], mybir.dt.bfloat16, tag="oh"
                        )
                        nc.vector.tensor_scalar(
                            oh[:], iota_f[:], srel_sb[:, col : col + 1],
                            None, op0=mybir.AluOpType.is_equal,
                        )
                        k += 1
                        nc.tensor.matmul(
                            psum[:], oh[:], ring[:, m % ring_sz, :],
                            start=(k == 1), stop=(k == n_mm),
                        )
                        col += 1

                rec = fin_pool.tile([128, 1], mybir.dt.float32, tag="rec")
                nc.vector.reciprocal(rec[:], cnt1_sb[:, g : g + 1])
                o_sb = fin_pool.tile([128, D], mybir.dt.bfloat16, tag="o")
                nc.vector.tensor_scalar_mul(o_sb[:], psum[:], rec[:])
                nc.sync.dma_start(out_d[g * 128 : (g + 1) * 128, :], o_sb[:])

    nc.compile()
    return nc


def _preprocess(features, neighbor_idx, segment_ids):
    """Host-side shard/index metadata construction (integers only)."""
    feat = np.ascontiguousarray(np.asarray(features, dtype=np.float32))
    seg = np.asarray(segment_ids).astype(np.int64)
    nid = np.asarray(neighbor_idx).astype(np.int64)
    n_edges = seg.shape[0]

    bf16 = mybir.dt.np(mybir.dt.bfloat16)
    featb = feat.astype(bf16)
    deg = np.bincount(seg, minlength=N_NODES)

    # two-range node sharding: per-core slices of the low and high dest
    # nodes, edge-balanced within each class
    e_low = int(np.searchsorted(seg, SPLIT))
    lowb = [0]
    for c in range(1, N_CORES):
        lowb.append(int(seg[min(c * e_low // N_CORES, max(e_low - 1, 0))]))
    lowb.append(SPLIT)
    highb = [SPLIT]
    for c in range(1, N_CORES):
        highb.append(
            int(seg[min(e_low + c * (n_edges - e_low) // N_CORES, n_edges - 1)])
        )
    highb.append(N_NODES)

    # per-core merged (regular + self-loop) edge lists in dest order, and
    # per-node class-split prefix sums; dest ids are core-relative with the
    # high slice appended after the low slice
    cores = []
    for c in range(N_CORES):
        nn_a = lowb[c + 1] - lowb[c]
        segs, xs = [], []
        for b0, b1, off in (
            (lowb[c], lowb[c + 1], 0),
            (highb[c], highb[c + 1], nn_a),
        ):
            lo, hi = np.searchsorted(seg, [b0, b1])
            nn = b1 - b0
            segs.append(
                np.concatenate([seg[lo:hi] - np.int64(b0), np.arange(nn)]) + off
            )
            xs.append(np.concatenate([nid[lo:hi], np.arange(b0, b1)]))
        s = np.concatenate(segs)
        x = np.concatenate(xs)
        order = np.argsort(s, kind="stable")
        s, x = s[order], x[order]
        nn = nn_a + (highb[c + 1] - highb[c])
        is_l = x < SPLIT
        cum_l = np.concatenate([[0], np.cumsum(np.bincount(s[is_l], minlength=nn))])
        cum_h = np.concatenate([[0], np.cumsum(np.bincount(s[~is_l], minlength=nn))])
        cores.append((s, x, nn_a, nn, cum_l, cum_h))

    # anchored slot packing: per core, choose <=128-node slot boundaries
    # tracking shared cumulative L/H stream targets so every core's slot-g
    # stream interval lands in (nearly) the same tiles
    node_bnds_all = []
    st_l = np.zeros((N_CORES, NS), np.int64)
    en_l = np.zeros((N_CORES, NS), np.int64)
    st_h = np.zeros((N_CORES, NS), np.int64)
    en_h = np.zeros((N_CORES, NS), np.int64)
    for c, (s, x, nn_a, nn, cum_l, cum_h) in enumerate(cores):
        node_bnds = [0]
        i = 0
        for g in range(NS):
            if g == NSA - 1:
                j = nn_a
            elif g == NS - 1:
                j = nn
            else:
                l_mid, h_mid = cum_l[nn_a], cum_h[nn_a]
                if g < NSA:
                    t_l = l_mid * (g + 1) / NSA
                    t_h = h_mid * (g + 1) / NSA
                    part_end = nn_a
                else:
                    t_l = l_mid + (cum_l[nn] - l_mid) * (g + 1 - NSA) / NSB
                    t_h = h_mid + (cum_h[nn] - h_mid) * (g + 1 - NSA) / NSB
                    part_end = nn
                js = np.arange(i + 1, min(i + 128, part_end) + 1)
                cost = np.abs(cum_l[js] - t_l) + np.abs(cum_h[js] - t_h)
                j = int(js[np.argmin(cost)])
            assert j - i <= 128
            st_l[c, g], en_l[c, g] = cum_l[i], cum_l[j]
            st_h[c, g], en_h[c, g] = cum_h[i], cum_h[j]
            node_bnds.append(j)
            i = j
        node_bnds_all.append(node_bnds)

    aL = (st_l.min(0) // 128).tolist()
    bL = (-(-en_l.max(0) // 128)).tolist()
    aH = (st_h.min(0) // 128).tolist()
    bH = (-(-en_h.max(0) // 128)).tolist()
    rows_l_act = -(-int(en_l.max(0)[-1]) // 16) * 16
    rows_h_act = -(-int(en_h.max(0)[-1]) // 16) * 16
    spans = (tuple(aL), tuple(bL), tuple(aH), tuple(bH), rows_l_act, rows_h_act)
    nt_l, nt_h = max(bL), max(bH)
    rows_l, rows_h = _pad_calls(nt_l * 128), _pad_calls(nt_h * 128)
    ncol = sum(b - a for a, b in zip(aL, bL)) + sum(
        b - a for a, b in zip(aH, bH)
    )

    in_maps = []
    slot_maps = []
    for c, (s, x, nn_a, nn, cum_l, cum_h) in enumerate(cores):
        node_bnds = node_bnds_all[c]
        is_l = x < SPLIT
        # dense class streams in dest order; within each slot's run, sort
        # by source row for HBM locality (order within a slot is free)
        xl, sl_ = x[is_l], s[is_l]
        xh, sh_ = x[~is_l] - SPLIT, s[~is_l]
        for g in range(NS):
            i, j = node_bnds[g], node_bnds[g + 1]
            for xs_, ss_, cum in ((xl, sl_, cum_l), (xh, sh_, cum_h)):
                a, b = int(cum[i]), int(cum[j])
                o = np.argsort(xs_[a:b], kind="stable")
                xs_[a:b], ss_[a:b] = xs_[a:b][o], ss_[a:b][o]

        gidx_all = np.zeros(rows_l + rows_h, np.int16)
        gidx_all[: len(xl)] = xl.astype(np.int16)
        gidx_all[rows_l : rows_l + len(xh)] = xh.astype(np.int16)

        srel_all = np.full((ncol, 128), -1.0, np.float32)
        cnt1 = np.ones((128, NS), np.float32)
        col = 0
        for g in range(NS):
            i, j = node_bnds[g], node_bnds[g + 1]
            for (a_t, b_t, st, en, ss_) in (
                (aL[g], bL[g], int(cum_l[i]), int(cum_l[j]), sl_),
                (aH[g], bH[g], int(cum_h[i]), int(cum_h[j]), sh_),
            ):
                for m in range(a_t, b_t):
                    r0, r1 = max(128 * m, st), min(128 * m + 128, en)
                    if r1 > r0:
                        srel_all[col, r0 - 128 * m : r1 - 128 * m] = (
                            ss_[r0:r1] - i
                        )
                    col += 1
            width = j - i
            if width:
                if i < nn_a:
                    abs_base = lowb[c] + i
                else:
                    abs_base = highb[c] + (i - nn_a)
                cnt1[:width, g] = 1.0 + deg[abs_base : abs_base + width]
        assert col == ncol

        gidx_w = np.ascontiguousarray(gidx_all.reshape(-1, 16).T)
        in_maps.append(
            {
                "featb": featb,
                "gidxi": gidx_w,
                "repmat": np.ascontiguousarray(
                    (np.arange(128)[None, :] % 16 == np.arange(16)[:, None])
                    .astype(np.float32)
                ),
                "srel": np.ascontiguousarray(
                    np.where(srel_all.T < 0, 255.0, srel_all.T)
                ).astype(np.uint8),
                "cnt1": cnt1,
            }
        )
        sm = []
        for g in range(NS):
            i, j = node_bnds[g], node_bnds[g + 1]
            if i < nn_a:
                sm.append((lowb[c] + i, j - i))
            else:
                sm.append((highb[c] + (i - nn_a), j - i))
        slot_maps.append(sm)
    return spans, in_maps, slot_maps


def kernel(features, neighbor_idx, segment_ids):
    global LAST_NC
    spans, in_maps, slot_maps = _preprocess(
        features, neighbor_idx, segment_ids
    )

    if spans not in _PROGRAM_CACHE:
        _PROGRAM_CACHE[spans] = _build_program(spans)
    nc = _PROGRAM_CACHE[spans]
    LAST_NC = nc

    try:
        res = bass_utils.run_bass_kernel_spmd(
            nc, in_maps, core_ids=list(range(N_CORES))
        )
    except Exception:
        # transient axon/device hiccups (e.g. recovering from a prior wedge)
        # have been observed to clear after a short pause
        import time

        time.sleep(20)
        res = bass_utils.run_bass_kernel_spmd(
            nc, in_maps, core_ids=list(range(N_CORES))
        )

    out = np.empty((N_NODES, D), np.float32)
    for c in range(N_CORES):
        oc = res.results[c]["out"].astype(np.float32)
        for g, (abs_base, width) in enumerate(slot_maps[c]):
            if width:
                out[abs_base : abs_base + width] = oc[g * 128 : g * 128 + width]
    return out
